# revision 1
# baseline (speedup 1.0000x reference)
"""CoverageLoss kernel for 8 Trainium2 NeuronCores.

Algorithm (per the retrieval_knn structure):
  loss = size(ls) + size(la) + cov(ss, ls) + cov(sa, la)
  cov(S, L): d = cdist_l1(S, L); sm4 = 4 smallest per row; tail = sm4.mean(-1)
             far = top64(tail); loss = mean(sm4[far]**2)

Device strategy (2D shard grid: 4 latent-shards x 2 sample-shards):
  Phase A (TensorEngine): quantized-L1 via thermometer encoding.
    With per-coord threshold grid t_q and crossing weights w_q,
    Dq(s,l) = sum_eq w_q * |1[s_e>t_q] - 1[l_e>t_q]|.  Encoding the sample
    side as s' = (1[s>t] - 1/2) and latent side as l' = w*1[l>t] gives
    <s', l'> = const(s-row) - Dq/2, so per-row argmax-8 of the matmul output
    directly yields the 8 approximately-nearest latents (InstMax/MaxIndex).
  Phase B (exact): indirect-DMA gather of the 8 candidate latent rows per
    sample; exact fp32 L1 distances via tensor_reduce(abs).  Only these 8
    values per (sample, shard) feed the loss, so the result is fp32-exact
    up to candidate selection (calibrated: <1e-5 rel err on the loss).
  Size losses: per-row relu(|x|_1 - 1)^2 on device; host means.
Host: slice/replicate shards in, merge 4x8 exact candidates per sample,
  top-64 selection over 2048 tails, final scalar.
"""

import os
from contextlib import ExitStack

import numpy as np

import concourse.bass as bass
import concourse.bacc as bacc
import concourse.mybir as mybir
import concourse.tile as tile
from concourse.bass_utils import run_bass_kernel_spmd

# ---- quantizer constants (Gaussian-quantile thresholds, Q=16) ----
THR = [-2.142141580581665, -1.5157124996185303, -1.161488652229309,
       -0.8928850293159485, -0.6660019755363464, -0.4625875651836395,
       -0.27278241515159607, -0.09017427265644073, 0.09017427265644073,
       0.27278241515159607, 0.4625875651836395, 0.6660019755363464,
       0.8928850293159485, 1.161488652229309, 1.5157124996185303,
       2.142141580581665]
W = [0.31321457028388977, 0.490326464176178, 0.3114137649536133,
     0.24774335324764252, 0.21514873206615448, 0.19660978019237518,
     0.18620665371418, 0.181478351354599, 0.181478351354599,
     0.18620665371418, 0.19660978019237518, 0.21514873206615448,
     0.24774335324764252, 0.3114137649536133, 0.490326464176178,
     0.31321457028388977]

Q = 16
NLAT, ES, EA = 8192, 64, 32
NSMP = 2048
A_SHARDS, B_SHARDS = 4, 2          # latent shards x sample shards
NL = NLAT // A_SHARDS              # 2048 latents per core
MS = NSMP // B_SHARDS              # 1024 samples per core
NTILES = MS // 128                 # 8 sample tiles
KC_S = ES * Q // 128               # 8 k-chunks (states: 64 coords x 2 thr)
KC_A = EA * Q // 128               # 4 k-chunks (actions: 32 coords x 4 thr)
NCHUNK = NL // 512                 # 4 psum column chunks
NCAND = 8

F32 = mybir.dt.float32
F16 = mybir.dt.float16
U32 = mybir.dt.uint32


def _cov_kernel(ctx, tc, e, kc, latTx, smpTx, lat_rows, smp_rows,
                refc_out, tag):
    """Emit one coverage pipeline (states or actions).

    latTx: [128, NL+2kc] f32: (128/e)-replicated coord-major transpose of
           latents, then kc threshold cols, then kc weight cols (appended so
           each encode op depends on exactly one input DMA: the TS ISA
           encoding has a single sync-wait slot).
    smpTx: [128, MS+2kc] f32, same layout for samples
    lat_rows:  [NL, e] f32 DRAM gather table
    smp_rows:  [MS, e] f32 DRAM sample rows
    refc_out:  [MS, NCAND] f32 DRAM exact candidate distances
    """
    nc = tc.nc
    enc = ctx.enter_context(tc.tile_pool(name=f"enc_{tag}", bufs=kc))
    psum = ctx.enter_context(tc.tile_pool(name=f"psum_{tag}", bufs=4,
                                          space="PSUM"))
    work = ctx.enter_context(tc.tile_pool(name=f"work_{tag}", bufs=2))
    small = ctx.enter_context(tc.tile_pool(name=f"small_{tag}", bufs=4))

    # ---- encode: one tensor_scalar per k-chunk per side ----
    bl = []
    bs = []
    for c in range(kc):
        blc = enc.tile([128, NL], F16, tag=f"bl_{tag}")
        nc.vector.tensor_scalar(
            out=blc[:], in0=latTx[:, :NL], scalar1=latTx[:, NL + c:NL + c + 1],
            scalar2=latTx[:, NL + kc + c:NL + kc + c + 1],
            op0=mybir.AluOpType.is_gt, op1=mybir.AluOpType.mult)
        bl.append(blc)
        bsc = enc.tile([128, MS], F16, tag=f"bs_{tag}")
        nc.vector.tensor_scalar(
            out=bsc[:], in0=smpTx[:, :MS], scalar1=smpTx[:, MS + c:MS + c + 1],
            scalar2=0.5, op0=mybir.AluOpType.is_gt,
            op1=mybir.AluOpType.subtract)
        bs.append(bsc)

    # all sample rows in one strided DMA: smp_big[p, m*e + j] = smp[m*128+p, j]
    smp_big = small.tile([128, NTILES * e], F32, tag=f"smpbig_{tag}")
    nc.sync.dma_start(
        smp_big[:], smp_rows.rearrange("(m p) e -> p m e", p=128))

    # ---- per sample-tile: matmul -> top8 -> gather -> exact refine ----
    for m in range(NTILES):
        smp_tile = smp_big[:, m * e:(m + 1) * e]

        dneg = work.tile([128, NL], F32, tag=f"dneg_{tag}")
        for n in range(NCHUNK):
            ps = psum.tile([128, 512], F32, tag=f"ps_{tag}")
            for k in range(kc):
                nc.tensor.matmul(
                    ps[:], lhsT=bs[k][:, m * 128:(m + 1) * 128],
                    rhs=bl[k][:, n * 512:(n + 1) * 512],
                    start=(k == 0), stop=(k == kc - 1))
            nc.scalar.copy(dneg[:, n * 512:(n + 1) * 512], ps[:])

        max8 = small.tile([128, 8], F32, tag=f"max8_{tag}")
        idx8 = small.tile([128, 8], U32, tag=f"idx8_{tag}")
        nc.vector.max(out=max8[:], in_=dneg[:])
        nc.vector.max_index(out=idx8[:], in_max=max8[:], in_values=dneg[:])

        gath = work.tile([128, NCAND * e], F32, tag=f"gath_{tag}")
        nc.gpsimd.indirect_dma_start(
            out=gath[:], out_offset=None, in_=lat_rows[:, :],
            in_offset=bass.IndirectOffsetOnAxis(ap=idx8[:, :], axis=0))

        diff = work.tile([128, NCAND * e], F32, tag=f"diff_{tag}")
        g3 = gath[:].rearrange("p (c e) -> p c e", c=NCAND)
        s3 = smp_tile[:, None, :].broadcast_to([128, NCAND, e])
        d3 = diff[:].rearrange("p (c e) -> p c e", c=NCAND)
        nc.vector.tensor_tensor(out=d3, in0=g3, in1=s3,
                                op=mybir.AluOpType.subtract)
        refc = small.tile([128, NCAND], F32, tag=f"refc_{tag}")
        nc.vector.tensor_reduce(
            out=refc[:], in_=d3, axis=mybir.AxisListType.X,
            op=mybir.AluOpType.add, apply_absolute_value=True)
        nc.sync.dma_start(refc_out[m * 128:(m + 1) * 128, :], refc[:])


def _size_kernel(ctx, tc, e, lat_rows, sz_out, tag):
    """Per-row relu(|x|_1 - 1)^2 for a [NL, e] latent shard -> sz_out [128, NL//128]."""
    nc = tc.nc
    pool = ctx.enter_context(tc.tile_pool(name=f"sz_{tag}", bufs=1))
    nt = NL // 128
    lat_big = pool.tile([128, nt * e], F32, tag=f"latbig_{tag}")
    nc.sync.dma_start(
        lat_big[:], lat_rows.rearrange("(m p) e -> p m e", p=128))
    norms = pool.tile([128, nt], F32, tag=f"norms_{tag}")
    nc.vector.tensor_reduce(
        out=norms[:], in_=lat_big[:].rearrange("p (m e) -> p m e", m=nt),
        axis=mybir.AxisListType.X, op=mybir.AluOpType.add,
        apply_absolute_value=True)
    rl = pool.tile([128, nt], F32, tag=f"rl_{tag}")
    nc.vector.tensor_scalar(out=rl[:], in0=norms[:], scalar1=1.0, scalar2=0.0,
                            op0=mybir.AluOpType.subtract,
                            op1=mybir.AluOpType.max)
    sq = pool.tile([128, nt], F32, tag=f"sq_{tag}")
    nc.vector.tensor_tensor(out=sq[:], in0=rl[:], in1=rl[:],
                            op=mybir.AluOpType.mult)
    nc.sync.dma_start(sz_out[:, :], sq[:])


def _build_nc():
    nc = bacc.Bacc("TRN2", target_bir_lowering=False, debug=False,
                   num_devices=8)
    inp = {}
    for name, shape in [
        ("latT2_s", [128, NL + 2 * KC_S]), ("latT4_a", [128, NL + 2 * KC_A]),
        ("smpT2_s", [128, MS + 2 * KC_S]), ("smpT4_a", [128, MS + 2 * KC_A]),
        ("lat_s", [NL, ES]), ("lat_a", [NL, EA]),
        ("smp_s", [MS, ES]), ("smp_a", [MS, EA]),
    ]:
        inp[name] = nc.dram_tensor(name, shape, F32, kind="ExternalInput").ap()
    out = {}
    for name, shape in [
        ("refc_s", [MS, NCAND]), ("refc_a", [MS, NCAND]),
        ("szrows_s", [128, NL // 128]), ("szrows_a", [128, NL // 128]),
    ]:
        out[name] = nc.dram_tensor(name, shape, F32, kind="ExternalOutput").ap()

    with tile.TileContext(nc) as tc:
        with ExitStack() as ctx:
            big = ctx.enter_context(tc.tile_pool(name="bigin", bufs=1))
            tiles = {}
            for name in ("latT2_s", "latT4_a", "smpT2_s", "smpT4_a"):
                t = big.tile(list(inp[name].shape), F32, tag=name)
                nc.sync.dma_start(t[:], inp[name][:, :])
                tiles[name] = t

            _cov_kernel(ctx, tc, ES, KC_S, tiles["latT2_s"][:],
                        tiles["smpT2_s"][:], inp["lat_s"],
                        inp["smp_s"], out["refc_s"], "s")
            _cov_kernel(ctx, tc, EA, KC_A, tiles["latT4_a"][:],
                        tiles["smpT4_a"][:], inp["lat_a"],
                        inp["smp_a"], out["refc_a"], "a")
            _size_kernel(ctx, tc, ES, inp["lat_s"], out["szrows_s"], "s")
            _size_kernel(ctx, tc, EA, inp["lat_a"], out["szrows_a"], "a")
    nc.compile()
    return nc


_NC_CACHE = {}


def _get_nc():
    if "nc" not in _NC_CACHE:
        _NC_CACHE["nc"] = _build_nc()
    return _NC_CACHE["nc"]


def _make_in_maps(latent_states, latent_actions, state_space_samples,
                  action_space_samples):
    thr = np.asarray(THR, np.float32)
    w = np.asarray(W, np.float32)
    # chunk c, partition p: states -> (coord p%64, thr 2c + p//64)
    thr_s = np.stack([np.repeat(thr[2 * c:2 * c + 2], 64) for c in range(KC_S)], 1)
    w_s = np.stack([np.repeat(w[2 * c:2 * c + 2], 64) for c in range(KC_S)], 1)
    thr_a = np.stack([np.repeat(thr[4 * c:4 * c + 4], 32) for c in range(KC_A)], 1)
    w_a = np.stack([np.repeat(w[4 * c:4 * c + 4], 32) for c in range(KC_A)], 1)
    tw_s = np.concatenate([thr_s, w_s], 1)
    tw_a = np.concatenate([thr_a, w_a], 1)

    in_maps = []
    for core in range(8):
        a, b = core % A_SHARDS, core // A_SHARDS
        lat_s = np.ascontiguousarray(latent_states[a * NL:(a + 1) * NL])
        lat_a = np.ascontiguousarray(latent_actions[a * NL:(a + 1) * NL])
        smp_s = np.ascontiguousarray(state_space_samples[b * MS:(b + 1) * MS])
        smp_a = np.ascontiguousarray(action_space_samples[b * MS:(b + 1) * MS])
        in_maps.append({
            "latT2_s": np.ascontiguousarray(
                np.concatenate([np.tile(lat_s.T, (2, 1)), tw_s], 1)),
            "latT4_a": np.ascontiguousarray(
                np.concatenate([np.tile(lat_a.T, (4, 1)), tw_a], 1)),
            "smpT2_s": np.ascontiguousarray(
                np.concatenate([np.tile(smp_s.T, (2, 1)), tw_s], 1)),
            "smpT4_a": np.ascontiguousarray(
                np.concatenate([np.tile(smp_a.T, (4, 1)), tw_a], 1)),
            "lat_s": lat_s, "lat_a": lat_a, "smp_s": smp_s, "smp_a": smp_a,
        })
    return in_maps


def _host_combine(results):
    """results: list of 8 per-core output dicts -> final scalar loss."""
    total = np.float64(0)
    # size losses: states from b=0 cores, actions from b=1 cores
    sz_s = [results[a]["szrows_s"] for a in range(A_SHARDS)]
    sz_a = [results[A_SHARDS + a]["szrows_a"] for a in range(A_SHARDS)]
    total += np.concatenate([s.ravel() for s in sz_s]).mean(dtype=np.float64)
    total += np.concatenate([s.ravel() for s in sz_a]).mean(dtype=np.float64)
    # coverage: merge per-shard exact candidate distances
    for key in ("refc_s", "refc_a"):
        ref = np.empty((NSMP, A_SHARDS * NCAND), np.float32)
        for core in range(8):
            a, b = core % A_SHARDS, core // A_SHARDS
            ref[b * MS:(b + 1) * MS, a * NCAND:(a + 1) * NCAND] = \
                results[core][key]
        ref.sort(axis=-1)
        sm4 = ref[:, :4]
        tails = sm4.mean(-1)
        far = np.argsort(-tails)[:64]
        total += np.float64((sm4[far].astype(np.float64) ** 2).mean())
    return np.float32(total)


def kernel(latent_states, latent_actions, state_space_samples,
           action_space_samples, _want_results=False, _trace=False):
    nc = _get_nc()
    in_maps = _make_in_maps(latent_states, latent_actions,
                            state_space_samples, action_space_samples)
    res = run_bass_kernel_spmd(nc, in_maps, core_ids=list(range(8)),
                               trace=_trace)
    out = _host_combine(res.results)
    if _want_results:
        return out, res
    return out



# revision 2
# speedup vs baseline: 2.4390x; 2.4390x over previous
"""CoverageLoss kernel for 8 Trainium2 NeuronCores — "ship-all" design.

Math: loss = size(ls) + size(la) + cov(ss, ls) + cov(sa, la)
  cov(S, L): d = cdist_l1(S, L); sm4 = 4 smallest per row; tail = sm4.mean(-1)
             far = top64(tail); loss = mean(sm4[far]**2)

Device strategy (4 latent shards x 2 sample shards = 8 cores):
  One-sided thermometer quantization: latents are snapped to a Q=8 Lloyd
  ladder (per-coordinate, Gaussian), samples stay EXACT.  For sample value s
  and quantized latent c_j, |s - c_j| is linear in the thermometer bits
  g_q = sign(l - t_q), so a single fp8 DoubleRow matmul
  M[s, l] = <u(s), g(l)> gives d_q(s, l) = A(s) + M[s, l] = sum_e |s_e - c(l_e)|
  exactly (up to fp8 rounding of u).  The device ships the entire quantized
  distance matrix back (uint8 with per-sample affine on the ACT engine /
  fp16 on DVE); the host does top-24 candidate selection, exact fp32 L1
  refinement, top-64 far selection, and the final scalar.  Size losses are
  host-side (trivially small).
"""

import numpy as np
import ml_dtypes
from contextlib import ExitStack

import concourse.bass as bass
import concourse.bacc as bacc
import concourse.mybir as mybir
import concourse.tile as tile
from concourse.bass_utils import run_bass_kernel_spmd

# ---- problem constants ----
NLAT, ES, EA = 8192, 64, 32
NSMP = 2048
TAIL, FAR = 4, 64
A_SHARDS, B_SHARDS = 4, 2
NL = NLAT // A_SHARDS              # 2048 latents per core
MS = NSMP // B_SHARDS              # 1024 samples per core
NTILES = MS // 128                 # 8 sample tiles

Q = 8                              # thermometer levels per coordinate
KC_S = ES * Q // 256               # 2 DoubleRow chunks (states)
KC_A = EA * Q // 256               # 1 DoubleRow chunk (actions)
NCHUNK = NL // 512                 # 4 psum column chunks

NCAND = 24                         # host-side candidate count per sample
U8_SCALE = 3.0                     # uint8 ladder: out = 250 - 3*d
U8_BIAS0 = 250.0

F32 = mybir.dt.float32
F16 = mybir.dt.float16
FP8 = mybir.dt.float8e4
U8 = mybir.dt.uint8

# tile-cov -> engine assignment: (cov, m) in ACT_SET goes ACT+uint8,
# the rest go DVE+fp16.  9 on ACT / 7 on DVE balances the two engines.
ACT_SET = {("s", 0), ("s", 2), ("s", 4), ("s", 6), ("s", 7),
           ("a", 1), ("a", 3), ("a", 5), ("a", 7)}


# ---- quantizer (host) ----
def _gauss_quantizer(Q):
    """Thresholds at Gaussian quantiles; ladder = Lloyd centroids."""
    from scipy.stats import norm
    qs = (np.arange(Q) + 0.5) / Q
    t = norm.ppf(qs)
    edges = np.concatenate([[-np.inf], t, [np.inf]])
    a, b = edges[:-1], edges[1:]
    c = (norm.pdf(a) - norm.pdf(b)) / np.maximum(norm.cdf(b) - norm.cdf(a), 1e-12)
    # solve-map:  |s - c_j| = A + sum_q u_q G[j, q],  G[j,q] = +1 if q<j else -1
    G = np.where(np.arange(Q)[None, :] < np.arange(Q + 1)[:, None], 1.0, -1.0)
    M = np.concatenate([np.ones((Q + 1, 1)), G], 1)
    return t.astype(np.float64), c.astype(np.float64), np.linalg.inv(M)


_T, _C, _MINV = _gauss_quantizer(Q)


def _encode_samples(S):
    """[n, e] samples -> (A_sum [n], U [n, e, Q] fp32 coefficient tensor)."""
    B = np.abs(S.astype(np.float64)[..., None] - _C)        # [n, e, Q+1]
    X = B @ _MINV.T                                          # [n, e, Q+1]
    A = X[..., 0].sum(-1)                                    # [n]
    U = X[..., 1:]                                           # [n, e, Q]
    return A.astype(np.float32), U.astype(np.float32)


def _encode_latents(L):
    """[N, e] latents -> g [N, e, Q] in {-1, +1}."""
    return np.where(L[:, :, None] > _T.astype(np.float32), 1.0, -1.0
                    ).astype(np.float32)


def _to_dr_layout(X, kc):
    """[rows, e, Q] -> [128, kc, 2, rows] fp8 DoubleRow layout.

    flat contraction index f = e*Q + q maps to (kc, ksub, kpart):
    f = kc*256 + ksub*128 + kpart.
    """
    rows = X.shape[0]
    Xf = X.reshape(rows, -1).T                               # [e*Q, rows]
    Xf = Xf.reshape(kc, 2, 128, rows).transpose(2, 0, 1, 3)  # [128, kc, 2, rows]
    return np.ascontiguousarray(Xf).astype(ml_dtypes.float8_e4m3)


# ---- device kernel ----
def _build_nc():
    nc = bacc.Bacc("TRN2", target_bir_lowering=False, debug=False,
                   num_devices=8)
    inp = {}
    for name, shape in [
        ("bl_s", [128, KC_S, 2, NL]), ("bs_s", [128, KC_S, 2, MS]),
        ("bl_a", [128, KC_A, 2, NL]), ("bs_a", [128, KC_A, 2, MS]),
    ]:
        inp[name] = nc.dram_tensor(name, shape, FP8, kind="ExternalInput").ap()
    for name in ("biasv_s", "biasv_a"):
        inp[name] = nc.dram_tensor(name, [128, NTILES], F32,
                                   kind="ExternalInput").ap()
    out = {}
    for name, shape, dt in [
        ("qd8_s", [128, NTILES, NL], U8), ("qd16_s", [128, NTILES, NL], F16),
        ("qd8_a", [128, NTILES, NL], U8), ("qd16_a", [128, NTILES, NL], F16),
    ]:
        out[name] = nc.dram_tensor(name, shape, dt, kind="ExternalOutput").ap()

    with tile.TileContext(nc) as tc:
        with ExitStack() as ctx:
            big = ctx.enter_context(tc.tile_pool(name="bigin", bufs=1))
            psum = ctx.enter_context(tc.tile_pool(name="psum", bufs=2,
                                                  space="PSUM"))
            st8 = ctx.enter_context(tc.tile_pool(name="st8", bufs=3))
            st16 = ctx.enter_context(tc.tile_pool(name="st16", bufs=3))

            enc = {}
            for name in ("bl_s", "bs_s", "bl_a", "bs_a"):
                t = big.tile(list(inp[name].shape), FP8, tag=name)
                nc.sync.dma_start(t[:], inp[name][:, :, :, :])
                enc[name] = t
            bias = {}
            for name in ("biasv_s", "biasv_a"):
                t = big.tile([128, NTILES], F32, tag=name)
                nc.sync.dma_start(t[:], inp[name][:, :])
                bias[name] = t

            covs = {
                "s": (KC_S, enc["bl_s"], enc["bs_s"], bias["biasv_s"],
                      out["qd8_s"], out["qd16_s"]),
                "a": (KC_A, enc["bl_a"], enc["bs_a"], bias["biasv_a"],
                      out["qd8_a"], out["qd16_a"]),
            }
            for m in range(NTILES):
                for cov in ("s", "a"):
                    kc_n, bl, bs, bv, o8, o16 = covs[cov]
                    ps = psum.tile([128, NL], F32, tag="ps")
                    for kc in range(kc_n):
                        lhsT = bs[:, kc, :, m * 128:(m + 1) * 128]
                        for n in range(NCHUNK):
                            nc.tensor.matmul(
                                ps[:, n * 512:(n + 1) * 512],
                                lhsT=lhsT,
                                rhs=bl[:, kc, :, n * 512:(n + 1) * 512],
                                start=(kc == 0), stop=(kc == kc_n - 1),
                                perf_mode=mybir.MatmulPerfMode.DoubleRow,
                                skip_group_check=True)
                    if (cov, m) in ACT_SET:
                        qt = st8.tile([128, NL], U8, tag="qt8")
                        nc.scalar.activation(
                            qt[:], ps[:], mybir.ActivationFunctionType.Relu,
                            bias=bv[:, m:m + 1], scale=-U8_SCALE)
                        nc.sync.dma_start(o8[:, m, :], qt[:])
                    else:
                        qt = st16.tile([128, NL], F16, tag="qt16")
                        nc.vector.tensor_copy(qt[:], ps[:])
                        nc.sync.dma_start(o16[:, m, :], qt[:])
    nc.compile()
    return nc


_NC_CACHE = {}


def _get_nc():
    if "nc" not in _NC_CACHE:
        _NC_CACHE["nc"] = _build_nc()
    return _NC_CACHE["nc"]


# ---- host pre/post ----
def _make_in_maps(latent_states, latent_actions, state_space_samples,
                  action_space_samples):
    g_s = _encode_latents(latent_states)       # [8192, 64, 8]
    g_a = _encode_latents(latent_actions)      # [8192, 32, 8]
    A_s, U_s = _encode_samples(state_space_samples)
    A_a, U_a = _encode_samples(action_space_samples)

    in_maps = []
    host = []                                  # per-core host context
    for core in range(8):
        a, b = core % A_SHARDS, core // A_SHARDS
        sl_l = slice(a * NL, (a + 1) * NL)
        sl_m = slice(b * MS, (b + 1) * MS)
        A_sb = A_s[sl_m]
        A_ab = A_a[sl_m]
        bias_s = (U8_BIAS0 - U8_SCALE *
                  A_sb.reshape(NTILES, 128).T).astype(np.float32)
        bias_a = (U8_BIAS0 - U8_SCALE *
                  A_ab.reshape(NTILES, 128).T).astype(np.float32)
        in_maps.append({
            "bl_s": _to_dr_layout(g_s[sl_l], KC_S),
            "bs_s": _to_dr_layout(U_s[sl_m], KC_S),
            "bl_a": _to_dr_layout(g_a[sl_l], KC_A),
            "bs_a": _to_dr_layout(U_a[sl_m], KC_A),
            "biasv_s": np.ascontiguousarray(bias_s),
            "biasv_a": np.ascontiguousarray(bias_a),
        })
        host.append({"a": a, "b": b})
    return in_maps, host


def _cov_loss_host(results, host, cov, samples, latents):
    """Assemble quantized rankings, exact-refine top candidates, compute
    the coverage loss term."""
    key8, key16 = f"qd8_{cov}", f"qd16_{cov}"
    sm4_all = np.empty((NSMP, TAIL), np.float32)
    for b in range(B_SHARDS):
        cores = [b * A_SHARDS + a for a in range(A_SHARDS)]
        # rank score: larger = closer.  uint8 tiles already are that;
        # fp16 tiles use -M.
        score = np.empty((MS, A_SHARDS * NL), np.float32)
        for a, c in enumerate(cores):
            r8 = results[c][key8]     # [128, NTILES, NL] uint8
            r16 = results[c][key16]   # [128, NTILES, NL] fp16
            for m in range(NTILES):
                rows = slice(m * 128, (m + 1) * 128)
                cols = slice(a * NL, (a + 1) * NL)
                if (cov, m) in ACT_SET:
                    score[rows, cols] = r8[:, m, :].astype(np.float32)
                else:
                    score[rows, cols] = -r16[:, m, :].astype(np.float32)
        idx = np.argpartition(-score, NCAND, axis=1)[:, :NCAND]
        smp = samples[b * MS:(b + 1) * MS]
        cand = latents[idx]                          # [MS, NCAND, e]
        d_ex = np.abs(smp[:, None, :] - cand).sum(-1, dtype=np.float32)
        d_ex.sort(axis=1)
        sm4_all[b * MS:(b + 1) * MS] = d_ex[:, :TAIL]
    tails = sm4_all.mean(-1)
    far = np.argsort(-tails)[:FAR]
    return float((sm4_all[far].astype(np.float64) ** 2).mean())


def _size_loss_host(latents):
    norms = np.abs(latents).sum(-1, dtype=np.float64)
    viol = np.maximum(norms - 1.0, 0.0)
    return float((viol ** 2).mean())


def kernel(latent_states, latent_actions, state_space_samples,
           action_space_samples, _want_results=False, _trace=False):
    latent_states = np.asarray(latent_states, np.float32)
    latent_actions = np.asarray(latent_actions, np.float32)
    state_space_samples = np.asarray(state_space_samples, np.float32)
    action_space_samples = np.asarray(action_space_samples, np.float32)

    nc = _get_nc()
    in_maps, host = _make_in_maps(latent_states, latent_actions,
                                  state_space_samples, action_space_samples)
    res = run_bass_kernel_spmd(nc, in_maps, core_ids=list(range(8)),
                               trace=_trace)
    total = np.float64(0)
    total += _size_loss_host(latent_states)
    total += _size_loss_host(latent_actions)
    total += _cov_loss_host(res.results, host, "s", state_space_samples,
                            latent_states)
    total += _cov_loss_host(res.results, host, "a", action_space_samples,
                            latent_actions)
    out = np.float32(total)
    if _want_results:
        return out, res
    return out


# revision 7
# speedup vs baseline: 2.5131x; 1.0304x over previous
"""CoverageLoss kernel for 8 Trainium2 NeuronCores — "ship-all" design.

Math: loss = size(ls) + size(la) + cov(ss, ls) + cov(sa, la)
  cov(S, L): d = cdist_l1(S, L); sm4 = 4 smallest per row; tail = sm4.mean(-1)
             far = top64(tail); loss = mean(sm4[far]**2)

Device strategy (4 latent shards x 2 sample shards = 8 cores):
  One-sided thermometer quantization: latents are snapped to a Q=8 Lloyd
  ladder (per-coordinate, Gaussian), samples stay EXACT.  For sample value s
  and quantized latent c_j, |s - c_j| is linear in the thermometer bits
  g_q = sign(l - t_q), so a single fp8 DoubleRow matmul
  M[s, l] = <u(s), g(l)> gives d_q(s, l) = A(s) + M[s, l] = sum_e |s_e - c(l_e)|
  exactly (up to fp8 rounding of u).  The device ships the entire quantized
  distance matrix back (uint8 with per-sample affine on the ACT engine /
  fp16 on DVE); the host does top-24 candidate selection, exact fp32 L1
  refinement, top-64 far selection, and the final scalar.  Size losses are
  host-side (trivially small).
"""

import numpy as np
import ml_dtypes
from contextlib import ExitStack

import concourse.bass as bass
import concourse.bacc as bacc
import concourse.mybir as mybir
import concourse.tile as tile
from concourse.bass_utils import run_bass_kernel_spmd

# ---- problem constants ----
NLAT, ES, EA = 8192, 64, 32
NSMP = 2048
TAIL, FAR = 4, 64
A_SHARDS, B_SHARDS = 4, 2
NL = NLAT // A_SHARDS              # 2048 latents per core
MS = NSMP // B_SHARDS              # 1024 samples per core
NTILES = MS // 128                 # 8 sample tiles

Q = 8                              # thermometer levels per coordinate
KC_S = ES * Q // 256               # 2 DoubleRow chunks (states)
KC_A = EA * Q // 256               # 1 DoubleRow chunk (actions)
NCHUNK = NL // 512                 # 4 psum column chunks

NCAND = 24                         # host-side candidate count per sample
U8_SCALE = 3.0                     # uint8 ladder: out = 250 - 3*d
U8_BIAS0 = 250.0

F32 = mybir.dt.float32
F16 = mybir.dt.float16
FP8 = mybir.dt.float8e4
U8 = mybir.dt.uint8

# tile-cov -> engine assignment: (cov, m) in ACT_SET goes to the scalar
# engine, the rest to DVE.  Both emit uint8 = relu(P + bias) where the
# PSUM value P = -3*M (the -3 is folded into the sample coefficients).
ACT_SET = {("s", 0), ("s", 2), ("s", 4), ("s", 6), ("s", 7),
           ("a", 1), ("a", 3), ("a", 5), ("a", 7)}


# ---- quantizer (host) ----
def _gauss_quantizer(Q):
    """Thresholds at Gaussian quantiles; ladder = Lloyd centroids."""
    from scipy.stats import norm
    qs = (np.arange(Q) + 0.5) / Q
    t = norm.ppf(qs)
    edges = np.concatenate([[-np.inf], t, [np.inf]])
    a, b = edges[:-1], edges[1:]
    c = (norm.pdf(a) - norm.pdf(b)) / np.maximum(norm.cdf(b) - norm.cdf(a), 1e-12)
    # solve-map:  |s - c_j| = A + sum_q u_q G[j, q],  G[j,q] = +1 if q<j else -1
    G = np.where(np.arange(Q)[None, :] < np.arange(Q + 1)[:, None], 1.0, -1.0)
    M = np.concatenate([np.ones((Q + 1, 1)), G], 1)
    return t.astype(np.float64), c.astype(np.float64), np.linalg.inv(M)


_T, _C, _MINV = _gauss_quantizer(Q)


def _encode_samples(S):
    """[n, e] samples -> (A_sum [n], U [n, e, Q] fp32 coefficient tensor)."""
    B = np.abs(S.astype(np.float64)[..., None] - _C)        # [n, e, Q+1]
    X = B @ _MINV.T                                          # [n, e, Q+1]
    A = X[..., 0].sum(-1)                                    # [n]
    U = X[..., 1:]                                           # [n, e, Q]
    return A.astype(np.float32), U.astype(np.float32)


def _encode_latents(L):
    """[N, e] latents -> g [N, e, Q] in {-1, +1}."""
    return np.where(L[:, :, None] > _T.astype(np.float32), 1.0, -1.0
                    ).astype(np.float32)


def _to_dr_layout(X, kc):
    """[rows, e, Q] -> [128, kc, 2, rows] fp8 DoubleRow layout.

    flat contraction index f = e*Q + q maps to (kc, ksub, kpart):
    f = kc*256 + ksub*128 + kpart.
    """
    rows = X.shape[0]
    Xf = X.reshape(rows, -1).T                               # [e*Q, rows]
    Xf = Xf.reshape(kc, 2, 128, rows).transpose(2, 0, 1, 3)  # [128, kc, 2, rows]
    return np.ascontiguousarray(Xf).astype(ml_dtypes.float8_e4m3)


# ---- device kernel ----
def _build_nc():
    nc = bacc.Bacc("TRN2", target_bir_lowering=False, debug=False,
                   num_devices=8)
    inp = {}
    for name, shape in [
        ("bl_s", [128, KC_S, 2, NL]), ("bs_s", [128, KC_S, 2, MS]),
        ("bl_a", [128, KC_A, 2, NL]), ("bs_a", [128, KC_A, 2, MS]),
    ]:
        inp[name] = nc.dram_tensor(name, shape, FP8, kind="ExternalInput").ap()
    for name in ("biasv_s", "biasv_a"):
        inp[name] = nc.dram_tensor(name, [128, NTILES], F32,
                                   kind="ExternalInput").ap()
    out = {}
    for name, shape, dt in [
        ("qd8_s", [128, NTILES, NL], U8),
        ("qd8_a", [128, NTILES, NL], U8),
    ]:
        out[name] = nc.dram_tensor(name, shape, dt, kind="ExternalOutput").ap()

    with tile.TileContext(nc) as tc:
        with ExitStack() as ctx:
            big = ctx.enter_context(tc.tile_pool(name="bigin", bufs=1))
            psum = ctx.enter_context(tc.tile_pool(name="psum", bufs=2,
                                                  space="PSUM"))
            st8 = ctx.enter_context(tc.tile_pool(name="st8", bufs=4))

            # biases first (small, needed by every consumer op)
            bias = {}
            for name in ("biasv_s", "biasv_a"):
                t = big.tile([128, NTILES], F32, tag=name)
                nc.sync.dma_start(t[:], inp[name][:, :])
                bias[name] = t
            # negated biases for the DVE path: out = (P - (-bias)) max 0
            nbias = {}
            for name in ("biasv_s", "biasv_a"):
                t = big.tile([128, NTILES], F32, tag=f"n{name}")
                nc.vector.tensor_scalar(
                    out=t[:], in0=bias[name][:], scalar1=-1.0, scalar2=None,
                    op0=mybir.AluOpType.mult)
                nbias[name] = t
            # encoded inputs, chunked so compute can start early
            enc = {}
            for name in ("bs_s", "bl_s", "bs_a", "bl_a"):
                t = big.tile(list(inp[name].shape), FP8, tag=name)
                kc_n = inp[name].shape[1]
                for kc in range(kc_n):
                    nc.sync.dma_start(t[:, kc, :, :], inp[name][:, kc, :, :])
                enc[name] = t

            # ACT table pre-warm (Relu) while DMAs stream
            warm8 = st8.tile([128, NL], U8, tag="qt8")
            nc.scalar.activation(
                warm8[:, 0:8], bias["biasv_s"][:, 0:8],
                mybir.ActivationFunctionType.Relu, bias=0.0, scale=1.0)
            # PE HAM warm-up: dummy matmuls on the first-arrived chunk
            psw = psum.tile([128, NL], F32, tag="ps")
            for _ in range(6):
                nc.tensor.matmul(
                    psw[:, 0:512],
                    lhsT=enc["bs_s"][:, 0, :, 0:128],
                    rhs=enc["bs_s"][:, 0, :, 0:512],
                    start=True, stop=True,
                    perf_mode=mybir.MatmulPerfMode.DoubleRow,
                    skip_group_check=True)

            covs = {
                "s": (KC_S, enc["bl_s"], enc["bs_s"], bias["biasv_s"],
                      nbias["biasv_s"], out["qd8_s"]),
                "a": (KC_A, enc["bl_a"], enc["bs_a"], bias["biasv_a"],
                      nbias["biasv_a"], out["qd8_a"]),
            }
            for m in range(NTILES):
                for cov in ("s", "a"):
                    kc_n, bl, bs, bv, nbv, o8 = covs[cov]
                    ps = psum.tile([128, NL], F32, tag="ps")
                    for kc in range(kc_n):
                        lhsT = bs[:, kc, :, m * 128:(m + 1) * 128]
                        for n in range(NCHUNK):
                            nc.tensor.matmul(
                                ps[:, n * 512:(n + 1) * 512],
                                lhsT=lhsT,
                                rhs=bl[:, kc, :, n * 512:(n + 1) * 512],
                                start=(kc == 0), stop=(kc == kc_n - 1),
                                perf_mode=mybir.MatmulPerfMode.DoubleRow,
                                skip_group_check=True)
                    qt = st8.tile([128, NL], U8, tag="qt8")
                    if (cov, m) in ACT_SET:
                        nc.scalar.activation(
                            qt[:], ps[:], mybir.ActivationFunctionType.Relu,
                            bias=bv[:, m:m + 1], scale=1.0)
                    else:
                        nc.vector.tensor_scalar(
                            out=qt[:], in0=ps[:], scalar1=nbv[:, m:m + 1],
                            scalar2=0.0, op0=mybir.AluOpType.subtract,
                            op1=mybir.AluOpType.max)
                    nc.sync.dma_start(o8[:, m, :], qt[:])
    nc.compile()
    return nc


_NC_CACHE = {}


def _get_nc():
    if "nc" not in _NC_CACHE:
        _NC_CACHE["nc"] = _build_nc()
    return _NC_CACHE["nc"]


# ---- host pre/post ----
def _make_in_maps(latent_states, latent_actions, state_space_samples,
                  action_space_samples):
    g_s = _encode_latents(latent_states)       # [8192, 64, 8]
    g_a = _encode_latents(latent_actions)      # [8192, 32, 8]
    A_s, U_s = _encode_samples(state_space_samples)
    A_a, U_a = _encode_samples(action_space_samples)

    in_maps = []
    host = []                                  # per-core host context
    for core in range(8):
        a, b = core % A_SHARDS, core // A_SHARDS
        sl_l = slice(a * NL, (a + 1) * NL)
        sl_m = slice(b * MS, (b + 1) * MS)
        A_sb = A_s[sl_m]
        A_ab = A_a[sl_m]
        bias_s = (U8_BIAS0 - U8_SCALE *
                  A_sb.reshape(NTILES, 128).T).astype(np.float32)
        bias_a = (U8_BIAS0 - U8_SCALE *
                  A_ab.reshape(NTILES, 128).T).astype(np.float32)
        # -U8_SCALE folded into the sample coefficients: PSUM P = -3*M
        in_maps.append({
            "bl_s": _to_dr_layout(g_s[sl_l], KC_S),
            "bs_s": _to_dr_layout(-U8_SCALE * U_s[sl_m], KC_S),
            "bl_a": _to_dr_layout(g_a[sl_l], KC_A),
            "bs_a": _to_dr_layout(-U8_SCALE * U_a[sl_m], KC_A),
            "biasv_s": np.ascontiguousarray(bias_s),
            "biasv_a": np.ascontiguousarray(bias_a),
        })
        host.append({"a": a, "b": b})
    return in_maps, host


def _cov_loss_host(results, host, cov, samples, latents):
    """Assemble quantized rankings, exact-refine top candidates, compute
    the coverage loss term."""
    key8 = f"qd8_{cov}"
    sm4_all = np.empty((NSMP, TAIL), np.float32)
    for b in range(B_SHARDS):
        cores = [b * A_SHARDS + a for a in range(A_SHARDS)]
        # rank score: larger = closer (uint8 = relu(250 - 3*d))
        score = np.empty((MS, A_SHARDS * NL), np.uint8)
        for a, c in enumerate(cores):
            r8 = results[c][key8]     # [128, NTILES, NL] uint8
            score[:, a * NL:(a + 1) * NL] = \
                r8.transpose(1, 0, 2).reshape(MS, NL)
        idx = np.argpartition(-score.astype(np.int16), NCAND,
                              axis=1)[:, :NCAND]
        smp = samples[b * MS:(b + 1) * MS]
        cand = latents[idx]                          # [MS, NCAND, e]
        d_ex = np.abs(smp[:, None, :] - cand).sum(-1, dtype=np.float32)
        d_ex.sort(axis=1)
        sm4_all[b * MS:(b + 1) * MS] = d_ex[:, :TAIL]
    tails = sm4_all.mean(-1)
    far = np.argsort(-tails)[:FAR]
    return float((sm4_all[far].astype(np.float64) ** 2).mean())


def _size_loss_host(latents):
    norms = np.abs(latents).sum(-1, dtype=np.float64)
    viol = np.maximum(norms - 1.0, 0.0)
    return float((viol ** 2).mean())


def kernel(latent_states, latent_actions, state_space_samples,
           action_space_samples, _want_results=False, _trace=False):
    latent_states = np.asarray(latent_states, np.float32)
    latent_actions = np.asarray(latent_actions, np.float32)
    state_space_samples = np.asarray(state_space_samples, np.float32)
    action_space_samples = np.asarray(action_space_samples, np.float32)

    nc = _get_nc()
    in_maps, host = _make_in_maps(latent_states, latent_actions,
                                  state_space_samples, action_space_samples)
    res = run_bass_kernel_spmd(nc, in_maps, core_ids=list(range(8)),
                               trace=_trace)
    total = np.float64(0)
    total += _size_loss_host(latent_states)
    total += _size_loss_host(latent_actions)
    total += _cov_loss_host(res.results, host, "s", state_space_samples,
                            latent_states)
    total += _cov_loss_host(res.results, host, "a", action_space_samples,
                            latent_actions)
    out = np.float32(total)
    if _want_results:
        return out, res
    return out


# revision 14
# speedup vs baseline: 2.9997x; 1.1936x over previous
"""CoverageLoss kernel for 8 Trainium2 NeuronCores — "ship-all" design.

Math: loss = size(ls) + size(la) + cov(ss, ls) + cov(sa, la)
  cov(S, L): d = cdist_l1(S, L); sm4 = 4 smallest per row; tail = sm4.mean(-1)
             far = top64(tail); loss = mean(sm4[far]**2)

Device strategy (4 latent shards x 2 sample shards = 8 cores):
  One-sided thermometer quantization: latents are snapped to a Q=8 Lloyd
  ladder (per-coordinate, Gaussian), samples stay EXACT.  For sample value s
  and quantized latent c_j, |s - c_j| is linear in the thermometer bits
  g_q = sign(l - t_q), so a single fp8 DoubleRow matmul
  M[s, l] = <u(s), g(l)> gives d_q(s, l) = A(s) + M[s, l] = sum_e |s_e - c(l_e)|
  exactly (up to fp8 rounding of u).  The device ships the entire quantized
  distance matrix back (uint8 with per-sample affine on the ACT engine /
  fp16 on DVE); the host does top-24 candidate selection, exact fp32 L1
  refinement, top-64 far selection, and the final scalar.  Size losses are
  host-side (trivially small).
"""

import numpy as np
import ml_dtypes
from contextlib import ExitStack

import concourse.bass as bass
import concourse.bacc as bacc
import concourse.mybir as mybir
import concourse.tile as tile
from concourse.bass_utils import run_bass_kernel_spmd

# ---- problem constants ----
NLAT, ES, EA = 8192, 64, 32
NSMP = 2048
TAIL, FAR = 4, 64
A_SHARDS, B_SHARDS = 4, 2
NL = NLAT // A_SHARDS              # 2048 latents per core
MS = NSMP // B_SHARDS              # 1024 samples per core
NTILES = MS // 128                 # 8 sample tiles

Q = 4                              # thermometer levels per coordinate
KSUB_S = 2                         # states: 256-deep DoubleRow chunks
KSUB_A = 1                         # actions: plain 128-deep fp8 chunks
KC_S = ES * Q // (128 * KSUB_S)    # 1 chunk (states)
KC_A = EA * Q // (128 * KSUB_A)    # 1 chunk (actions)
NCHUNK = NL // 512                 # 4 psum column chunks

NCAND = 64                         # host-side candidate count per sample
U8_SCALE = 3.0                     # uint8 ladder: out = 250 - 3*d
U8_BIAS0 = 250.0

F32 = mybir.dt.float32
F16 = mybir.dt.float16
FP8 = mybir.dt.float8e4
U8 = mybir.dt.uint8

# tile-cov -> engine assignment: (cov, m) in ACT_SET goes to the scalar
# engine, the rest to DVE.  Both emit uint8 = relu(P + bias) where the
# PSUM value P = -3*M (the -3 is folded into the sample coefficients).
ACT_SET = {("s", 0), ("s", 2), ("s", 4), ("s", 6), ("s", 7),
           ("a", 1), ("a", 3), ("a", 5), ("a", 7)}


# ---- quantizer (host) ----
def _gauss_quantizer(Q):
    """Thresholds at Gaussian quantiles; ladder = Lloyd centroids."""
    from scipy.stats import norm
    qs = (np.arange(Q) + 0.5) / Q
    t = norm.ppf(qs)
    edges = np.concatenate([[-np.inf], t, [np.inf]])
    a, b = edges[:-1], edges[1:]
    c = (norm.pdf(a) - norm.pdf(b)) / np.maximum(norm.cdf(b) - norm.cdf(a), 1e-12)
    # solve-map:  |s - c_j| = A + sum_q u_q G[j, q],  G[j,q] = +1 if q<j else -1
    G = np.where(np.arange(Q)[None, :] < np.arange(Q + 1)[:, None], 1.0, -1.0)
    M = np.concatenate([np.ones((Q + 1, 1)), G], 1)
    return t.astype(np.float64), c.astype(np.float64), np.linalg.inv(M)


_T, _C, _MINV = _gauss_quantizer(Q)


def _encode_samples(S):
    """[n, e] samples -> (A_sum [n], U [n, e, Q] fp32 coefficient tensor)."""
    B = np.abs(S.astype(np.float64)[..., None] - _C)        # [n, e, Q+1]
    X = B @ _MINV.T                                          # [n, e, Q+1]
    A = X[..., 0].sum(-1)                                    # [n]
    U = X[..., 1:]                                           # [n, e, Q]
    return A.astype(np.float32), U.astype(np.float32)


def _encode_latents(L):
    """[N, e] latents -> g [N, e, Q] in {-1, +1}."""
    return np.where(L[:, :, None] > _T.astype(np.float32), 1.0, -1.0
                    ).astype(np.float32)


def _to_dr_layout(X, kc, ksub):
    """[rows, e, Q] -> [128, kc, ksub, rows] fp8 layout.

    flat contraction index f = e*Q + q maps to (kc, ksub, kpart):
    f = ((kc*ksub) + ks)*128 + kpart.
    """
    rows = X.shape[0]
    Xf = X.reshape(rows, -1).T                               # [e*Q, rows]
    Xf = Xf.reshape(kc, ksub, 128, rows).transpose(2, 0, 1, 3)
    return np.ascontiguousarray(Xf).astype(ml_dtypes.float8_e4m3)


# ---- device kernel ----
def _build_nc():
    nc = bacc.Bacc("TRN2", target_bir_lowering=False, debug=False,
                   num_devices=8)
    inp = {}
    for name, shape in [
        ("bl_s", [128, KC_S, KSUB_S, NL]), ("bs_s", [128, KC_S, KSUB_S, MS]),
        ("bl_a", [128, KC_A, KSUB_A, NL]), ("bs_a", [128, KC_A, KSUB_A, MS]),
    ]:
        inp[name] = nc.dram_tensor(name, shape, FP8, kind="ExternalInput").ap()
    for name in ("biasv_s", "biasv_a"):
        inp[name] = nc.dram_tensor(name, [128, NTILES], F32,
                                   kind="ExternalInput").ap()
    out = {}
    for name, shape, dt in [
        ("qd8_s", [128, NTILES, NL], U8),
        ("qd8_a", [128, NTILES, NL], U8),
    ]:
        out[name] = nc.dram_tensor(name, shape, dt, kind="ExternalOutput").ap()

    with tile.TileContext(nc) as tc:
        with ExitStack() as ctx:
            big = ctx.enter_context(tc.tile_pool(name="bigin", bufs=1))
            psum = ctx.enter_context(tc.tile_pool(name="psum", bufs=2,
                                                  space="PSUM"))
            st8 = ctx.enter_context(tc.tile_pool(name="st8", bufs=4))

            # biases first (small, needed by every consumer op)
            bias = {}
            for name in ("biasv_s", "biasv_a"):
                t = big.tile([128, NTILES], F32, tag=name)
                nc.sync.dma_start(t[:], inp[name][:, :])
                bias[name] = t
            # negated biases for the DVE path: out = (P - (-bias)) max 0
            nbias = {}
            for name in ("biasv_s", "biasv_a"):
                t = big.tile([128, NTILES], F32, tag=f"n{name}")
                nc.vector.tensor_scalar(
                    out=t[:], in0=bias[name][:], scalar1=-1.0, scalar2=None,
                    op0=mybir.AluOpType.mult)
                nbias[name] = t
            # encoded inputs, chunked so compute can start early
            enc = {}
            for name in ("bs_s", "bl_s", "bs_a", "bl_a"):
                t = big.tile(list(inp[name].shape), FP8, tag=name)
                kc_n = inp[name].shape[1]
                for kc in range(kc_n):
                    nc.sync.dma_start(t[:, kc, :, :], inp[name][:, kc, :, :])
                enc[name] = t

            # ACT table pre-warm (Relu) while DMAs stream
            warm8 = st8.tile([128, NL], U8, tag="qt8")
            nc.scalar.activation(
                warm8[:, 0:8], bias["biasv_s"][:, 0:8],
                mybir.ActivationFunctionType.Relu, bias=0.0, scale=1.0)
            # PE HAM warm-up: dummy matmuls on the first-arrived chunk
            psw = psum.tile([128, NL], F32, tag="ps")
            for _ in range(6):
                nc.tensor.matmul(
                    psw[:, 0:512],
                    lhsT=enc["bs_s"][:, 0, :, 0:128],
                    rhs=enc["bs_s"][:, 0, :, 0:512],
                    start=True, stop=True,
                    perf_mode=mybir.MatmulPerfMode.DoubleRow,
                    skip_group_check=True)

            covs = {
                "s": (KC_S, KSUB_S, enc["bl_s"], enc["bs_s"], bias["biasv_s"],
                      nbias["biasv_s"], out["qd8_s"]),
                "a": (KC_A, KSUB_A, enc["bl_a"], enc["bs_a"], bias["biasv_a"],
                      nbias["biasv_a"], out["qd8_a"]),
            }
            for m in range(NTILES):
                for cov in ("s", "a"):
                    kc_n, ksub, bl, bs, bv, nbv, o8 = covs[cov]
                    dr = (mybir.MatmulPerfMode.DoubleRow if ksub == 2
                          else None)
                    ps = psum.tile([128, NL], F32, tag="ps")
                    for kc in range(kc_n):
                        if ksub == 2:
                            lhsT = bs[:, kc, :, m * 128:(m + 1) * 128]
                        else:
                            lhsT = bs[:, kc, 0, m * 128:(m + 1) * 128]
                        for n in range(NCHUNK):
                            rhs = (bl[:, kc, :, n * 512:(n + 1) * 512]
                                   if ksub == 2 else
                                   bl[:, kc, 0, n * 512:(n + 1) * 512])
                            nc.tensor.matmul(
                                ps[:, n * 512:(n + 1) * 512],
                                lhsT=lhsT, rhs=rhs,
                                start=(kc == 0), stop=(kc == kc_n - 1),
                                perf_mode=dr,
                                skip_group_check=True)
                    qt = st8.tile([128, NL], U8, tag="qt8")
                    if (cov, m) in ACT_SET:
                        nc.scalar.activation(
                            qt[:], ps[:], mybir.ActivationFunctionType.Relu,
                            bias=bv[:, m:m + 1], scale=1.0)
                    else:
                        nc.vector.tensor_scalar(
                            out=qt[:], in0=ps[:], scalar1=nbv[:, m:m + 1],
                            scalar2=0.0, op0=mybir.AluOpType.subtract,
                            op1=mybir.AluOpType.max)
                    nc.sync.dma_start(o8[:, m, :], qt[:])
    nc.compile()
    return nc


_NC_CACHE = {}


def _get_nc():
    if "nc" not in _NC_CACHE:
        _NC_CACHE["nc"] = _build_nc()
    return _NC_CACHE["nc"]


# ---- host pre/post ----
def _make_in_maps(latent_states, latent_actions, state_space_samples,
                  action_space_samples):
    g_s = _encode_latents(latent_states)       # [8192, 64, 8]
    g_a = _encode_latents(latent_actions)      # [8192, 32, 8]
    A_s, U_s = _encode_samples(state_space_samples)
    A_a, U_a = _encode_samples(action_space_samples)

    in_maps = []
    host = []                                  # per-core host context
    for core in range(8):
        a, b = core % A_SHARDS, core // A_SHARDS
        sl_l = slice(a * NL, (a + 1) * NL)
        sl_m = slice(b * MS, (b + 1) * MS)
        A_sb = A_s[sl_m]
        A_ab = A_a[sl_m]
        bias_s = (U8_BIAS0 - U8_SCALE *
                  A_sb.reshape(NTILES, 128).T).astype(np.float32)
        bias_a = (U8_BIAS0 - U8_SCALE *
                  A_ab.reshape(NTILES, 128).T).astype(np.float32)
        # -U8_SCALE folded into the sample coefficients: PSUM P = -3*M
        in_maps.append({
            "bl_s": _to_dr_layout(g_s[sl_l], KC_S, KSUB_S),
            "bs_s": _to_dr_layout(-U8_SCALE * U_s[sl_m], KC_S, KSUB_S),
            "bl_a": _to_dr_layout(g_a[sl_l], KC_A, KSUB_A),
            "bs_a": _to_dr_layout(-U8_SCALE * U_a[sl_m], KC_A, KSUB_A),
            "biasv_s": np.ascontiguousarray(bias_s),
            "biasv_a": np.ascontiguousarray(bias_a),
        })
        host.append({"a": a, "b": b})
    return in_maps, host


def _cov_loss_host(results, host, cov, samples, latents):
    """Assemble quantized rankings, exact-refine top candidates, compute
    the coverage loss term."""
    key8 = f"qd8_{cov}"
    sm4_all = np.empty((NSMP, TAIL), np.float32)
    for b in range(B_SHARDS):
        cores = [b * A_SHARDS + a for a in range(A_SHARDS)]
        # rank score: larger = closer (uint8 = relu(250 - 3*d))
        score = np.empty((MS, A_SHARDS * NL), np.uint8)
        for a, c in enumerate(cores):
            r8 = results[c][key8]     # [128, NTILES, NL] uint8
            score[:, a * NL:(a + 1) * NL] = \
                r8.transpose(1, 0, 2).reshape(MS, NL)
        idx = np.argpartition(-score.astype(np.int16), NCAND,
                              axis=1)[:, :NCAND]
        smp = samples[b * MS:(b + 1) * MS]
        cand = latents[idx]                          # [MS, NCAND, e]
        d_ex = np.abs(smp[:, None, :] - cand).sum(-1, dtype=np.float32)
        d_ex.sort(axis=1)
        sm4_all[b * MS:(b + 1) * MS] = d_ex[:, :TAIL]
    tails = sm4_all.mean(-1)
    far = np.argsort(-tails)[:FAR]
    return float((sm4_all[far].astype(np.float64) ** 2).mean())


def _size_loss_host(latents):
    norms = np.abs(latents).sum(-1, dtype=np.float64)
    viol = np.maximum(norms - 1.0, 0.0)
    return float((viol ** 2).mean())


def kernel(latent_states, latent_actions, state_space_samples,
           action_space_samples, _want_results=False, _trace=False):
    latent_states = np.asarray(latent_states, np.float32)
    latent_actions = np.asarray(latent_actions, np.float32)
    state_space_samples = np.asarray(state_space_samples, np.float32)
    action_space_samples = np.asarray(action_space_samples, np.float32)

    nc = _get_nc()
    in_maps, host = _make_in_maps(latent_states, latent_actions,
                                  state_space_samples, action_space_samples)
    res = run_bass_kernel_spmd(nc, in_maps, core_ids=list(range(8)),
                               trace=_trace)
    total = np.float64(0)
    total += _size_loss_host(latent_states)
    total += _size_loss_host(latent_actions)
    total += _cov_loss_host(res.results, host, "s", state_space_samples,
                            latent_states)
    total += _cov_loss_host(res.results, host, "a", action_space_samples,
                            latent_actions)
    out = np.float32(total)
    if _want_results:
        return out, res
    return out


# revision 21
# speedup vs baseline: 3.0092x; 1.0032x over previous
"""CoverageLoss kernel for 8 Trainium2 NeuronCores — "ship-all" design.

Math: loss = size(ls) + size(la) + cov(ss, ls) + cov(sa, la)
  cov(S, L): d = cdist_l1(S, L); sm4 = 4 smallest per row; tail = sm4.mean(-1)
             far = top64(tail); loss = mean(sm4[far]**2)

Device strategy (4 latent shards x 2 sample shards = 8 cores):
  One-sided thermometer quantization: latents are snapped to a Q=8 Lloyd
  ladder (per-coordinate, Gaussian), samples stay EXACT.  For sample value s
  and quantized latent c_j, |s - c_j| is linear in the thermometer bits
  g_q = sign(l - t_q), so a single fp8 DoubleRow matmul
  M[s, l] = <u(s), g(l)> gives d_q(s, l) = A(s) + M[s, l] = sum_e |s_e - c(l_e)|
  exactly (up to fp8 rounding of u).  The device ships the entire quantized
  distance matrix back (uint8 with per-sample affine on the ACT engine /
  fp16 on DVE); the host does top-24 candidate selection, exact fp32 L1
  refinement, top-64 far selection, and the final scalar.  Size losses are
  host-side (trivially small).
"""

import numpy as np
import ml_dtypes
from contextlib import ExitStack

import concourse.bass as bass
import concourse.bacc as bacc
import concourse.mybir as mybir
import concourse.tile as tile
from concourse.bass_utils import run_bass_kernel_spmd

# ---- problem constants ----
NLAT, ES, EA = 8192, 64, 32
NSMP = 2048
TAIL, FAR = 4, 64
A_SHARDS, B_SHARDS = 4, 2
NL = NLAT // A_SHARDS              # 2048 latents per core
MS = NSMP // B_SHARDS              # 1024 samples per core
NTILES = MS // 128                 # 8 sample tiles

Q_S = 2                            # thermometer levels per state coordinate
Q_A = 4                            # thermometer levels per action coordinate
KSUB_S = 1                         # plain 128-deep fp8 chunks
KSUB_A = 1
KC_S = ES * Q_S // (128 * KSUB_S)  # 1 chunk (states)
KC_A = EA * Q_A // (128 * KSUB_A)  # 1 chunk (actions)
NCHUNK = NL // 512                 # 4 psum column chunks

NCAND = 128                        # host-side candidate count per sample
U8_SCALE = 3.0                     # uint8 ladder: out = 250 - 3*d
U8_BIAS0 = 250.0

F32 = mybir.dt.float32
F16 = mybir.dt.float16
FP8 = mybir.dt.float8e4
U8 = mybir.dt.uint8

# tile-cov -> engine assignment: (cov, m) in ACT_SET goes to the scalar
# engine, the rest to DVE.  Both emit uint8 = relu(P + bias) where the
# PSUM value P = -3*M (the -3 is folded into the sample coefficients).
ACT_SET = {("s", 0), ("s", 2), ("s", 4), ("s", 6), ("s", 7),
           ("a", 1), ("a", 3), ("a", 5), ("a", 7)}


# ---- quantizer (host) ----
def _gauss_quantizer(Q):
    """Thresholds at Gaussian quantiles; ladder = Lloyd centroids."""
    from scipy.stats import norm
    qs = (np.arange(Q) + 0.5) / Q
    t = norm.ppf(qs)
    edges = np.concatenate([[-np.inf], t, [np.inf]])
    a, b = edges[:-1], edges[1:]
    c = (norm.pdf(a) - norm.pdf(b)) / np.maximum(norm.cdf(b) - norm.cdf(a), 1e-12)
    # solve-map:  |s - c_j| = A + sum_q u_q G[j, q],  G[j,q] = +1 if q<j else -1
    G = np.where(np.arange(Q)[None, :] < np.arange(Q + 1)[:, None], 1.0, -1.0)
    M = np.concatenate([np.ones((Q + 1, 1)), G], 1)
    return t.astype(np.float64), c.astype(np.float64), np.linalg.inv(M)


_QZ = {q: _gauss_quantizer(q) for q in {Q_S, Q_A}}


def _encode_samples(S, Q):
    """[n, e] samples -> (A_sum [n], U [n, e, Q] fp32 coefficient tensor)."""
    t, c, minv = _QZ[Q]
    B = np.abs(S.astype(np.float64)[..., None] - c)         # [n, e, Q+1]
    X = B @ minv.T                                           # [n, e, Q+1]
    A = X[..., 0].sum(-1)                                    # [n]
    U = X[..., 1:]                                           # [n, e, Q]
    return A.astype(np.float32), U.astype(np.float32)


def _encode_latents(L, Q):
    """[N, e] latents -> g [N, e, Q] in {-1, +1}."""
    t, c, minv = _QZ[Q]
    return np.where(L[:, :, None] > t.astype(np.float32), 1.0, -1.0
                    ).astype(np.float32)


def _to_dr_layout(X, kc, ksub):
    """[rows, e, Q] -> [128, kc, ksub, rows] fp8 layout.

    flat contraction index f = e*Q + q maps to (kc, ksub, kpart):
    f = ((kc*ksub) + ks)*128 + kpart.
    """
    rows = X.shape[0]
    Xf = X.reshape(rows, -1).T                               # [e*Q, rows]
    Xf = Xf.reshape(kc, ksub, 128, rows).transpose(2, 0, 1, 3)
    return np.ascontiguousarray(Xf).astype(ml_dtypes.float8_e4m3)


# ---- device kernel ----
def _build_nc():
    nc = bacc.Bacc("TRN2", target_bir_lowering=False, debug=False,
                   num_devices=8)
    inp = {}
    for name, shape in [
        ("bl_s", [128, KC_S, KSUB_S, NL]), ("bs_s", [128, KC_S, KSUB_S, MS]),
        ("bl_a", [128, KC_A, KSUB_A, NL]), ("bs_a", [128, KC_A, KSUB_A, MS]),
    ]:
        inp[name] = nc.dram_tensor(name, shape, FP8, kind="ExternalInput").ap()
    for name in ("biasv_s", "biasv_a"):
        inp[name] = nc.dram_tensor(name, [128, NTILES], F32,
                                   kind="ExternalInput").ap()
    inp["warmT"] = nc.dram_tensor("warmT", [128, 512], FP8,
                                  kind="ExternalInput").ap()
    qd8 = nc.dram_tensor("qd8", [128, NTILES, 2, NL], U8,
                         kind="ExternalOutput").ap()

    with tile.TileContext(nc) as tc:
        with ExitStack() as ctx:
            big = ctx.enter_context(tc.tile_pool(name="bigin", bufs=1))
            psum = ctx.enter_context(tc.tile_pool(name="psum", bufs=2,
                                                  space="PSUM"))
            st8 = ctx.enter_context(tc.tile_pool(name="st8", bufs=4))

            # warm-up weights first: tiny DMA, PE dummies run during the
            # big input DMAs so HAM is at 8/8 when real matmuls start
            warmt = big.tile([128, 512], FP8, tag="warmT")
            nc.sync.dma_start(warmt[:], inp["warmT"][:, :])
            psw = psum.tile([128, NL], F32, tag="ps")
            for _ in range(10):
                nc.tensor.matmul(
                    psw[:, 0:512], lhsT=warmt[:, 0:128], rhs=warmt[:, 0:512],
                    start=True, stop=True, skip_group_check=True)

            # biases (small, needed by every consumer op)
            bias = {}
            for name in ("biasv_s", "biasv_a"):
                t = big.tile([128, NTILES], F32, tag=name)
                nc.sync.dma_start(t[:], inp[name][:, :])
                bias[name] = t
            # negated biases for the DVE path: out = (P - (-bias)) max 0
            nbias = {}
            for name in ("biasv_s", "biasv_a"):
                t = big.tile([128, NTILES], F32, tag=f"n{name}")
                nc.vector.tensor_scalar(
                    out=t[:], in0=bias[name][:], scalar1=-1.0, scalar2=None,
                    op0=mybir.AluOpType.mult)
                nbias[name] = t
            # encoded inputs, split across DMA issue engines
            enc = {}
            for name, eng in (("bs_s", nc.sync), ("bl_s", nc.sync),
                              ("bs_a", nc.gpsimd), ("bl_a", nc.gpsimd)):
                t = big.tile(list(inp[name].shape), FP8, tag=name)
                eng.dma_start(t[:, 0, :, :], inp[name][:, 0, :, :])
                enc[name] = t

            # ACT table pre-warm (Relu) while DMAs stream
            warm8 = st8.tile([128, 2, NL], U8, tag="qt8")
            nc.scalar.activation(
                warm8[:, 0, 0:8], bias["biasv_s"][:, 0:8],
                mybir.ActivationFunctionType.Relu, bias=0.0, scale=1.0)

            covs = {
                "s": (enc["bl_s"], enc["bs_s"], bias["biasv_s"],
                      nbias["biasv_s"]),
                "a": (enc["bl_a"], enc["bs_a"], bias["biasv_a"],
                      nbias["biasv_a"]),
            }
            for m in range(NTILES):
                qt = st8.tile([128, 2, NL], U8, tag="qt8")
                for ci, cov in enumerate(("s", "a")):
                    bl, bs, bv, nbv = covs[cov]
                    ps = psum.tile([128, NL], F32, tag="ps")
                    lhsT = bs[:, 0, 0, m * 128:(m + 1) * 128]
                    for n in range(NCHUNK):
                        nc.tensor.matmul(
                            ps[:, n * 512:(n + 1) * 512],
                            lhsT=lhsT,
                            rhs=bl[:, 0, 0, n * 512:(n + 1) * 512],
                            start=True, stop=True,
                            skip_group_check=True)
                    if (cov, m) in ACT_SET:
                        nc.scalar.activation(
                            qt[:, ci, :], ps[:],
                            mybir.ActivationFunctionType.Relu,
                            bias=bv[:, m:m + 1], scale=1.0)
                    else:
                        nc.vector.tensor_scalar(
                            out=qt[:, ci, :], in0=ps[:],
                            scalar1=nbv[:, m:m + 1],
                            scalar2=0.0, op0=mybir.AluOpType.subtract,
                            op1=mybir.AluOpType.max)
                eng = nc.sync if m % 2 == 0 else nc.gpsimd
                eng.dma_start(qd8[:, m, :, :], qt[:])
    nc.compile()
    return nc


_NC_CACHE = {}


def _get_nc():
    if "nc" not in _NC_CACHE:
        _NC_CACHE["nc"] = _build_nc()
    return _NC_CACHE["nc"]


# ---- host pre/post ----
def _make_in_maps(latent_states, latent_actions, state_space_samples,
                  action_space_samples):
    g_s = _encode_latents(latent_states, Q_S)   # [8192, 64, Q_S]
    g_a = _encode_latents(latent_actions, Q_A)  # [8192, 32, Q_A]
    A_s, U_s = _encode_samples(state_space_samples, Q_S)
    A_a, U_a = _encode_samples(action_space_samples, Q_A)
    warm = np.full((128, 512), 0.25, ml_dtypes.float8_e4m3)

    in_maps = []
    host = []                                  # per-core host context
    for core in range(8):
        a, b = core % A_SHARDS, core // A_SHARDS
        sl_l = slice(a * NL, (a + 1) * NL)
        sl_m = slice(b * MS, (b + 1) * MS)
        A_sb = A_s[sl_m]
        A_ab = A_a[sl_m]
        bias_s = (U8_BIAS0 - U8_SCALE *
                  A_sb.reshape(NTILES, 128).T).astype(np.float32)
        bias_a = (U8_BIAS0 - U8_SCALE *
                  A_ab.reshape(NTILES, 128).T).astype(np.float32)
        # -U8_SCALE folded into the sample coefficients: PSUM P = -3*M
        in_maps.append({
            "bl_s": _to_dr_layout(g_s[sl_l], KC_S, KSUB_S),
            "bs_s": _to_dr_layout(-U8_SCALE * U_s[sl_m], KC_S, KSUB_S),
            "bl_a": _to_dr_layout(g_a[sl_l], KC_A, KSUB_A),
            "bs_a": _to_dr_layout(-U8_SCALE * U_a[sl_m], KC_A, KSUB_A),
            "biasv_s": np.ascontiguousarray(bias_s),
            "biasv_a": np.ascontiguousarray(bias_a),
            "warmT": warm,
        })
        host.append({"a": a, "b": b})
    return in_maps, host


def _cov_loss_host(results, host, cov, samples, latents):
    """Assemble quantized rankings, exact-refine top candidates, compute
    the coverage loss term."""
    ci = 0 if cov == "s" else 1
    sm4_all = np.empty((NSMP, TAIL), np.float32)
    for b in range(B_SHARDS):
        cores = [b * A_SHARDS + a for a in range(A_SHARDS)]
        # rank score: larger = closer (uint8 = relu(250 - 3*d))
        score = np.empty((MS, A_SHARDS * NL), np.uint8)
        for a, c in enumerate(cores):
            r8 = results[c]["qd8"]    # [128, NTILES, 2, NL] uint8
            score[:, a * NL:(a + 1) * NL] = \
                r8[:, :, ci, :].transpose(1, 0, 2).reshape(MS, NL)
        idx = np.argpartition(-score.astype(np.int16), NCAND,
                              axis=1)[:, :NCAND]
        smp = samples[b * MS:(b + 1) * MS]
        cand = latents[idx]                          # [MS, NCAND, e]
        d_ex = np.abs(smp[:, None, :] - cand).sum(-1, dtype=np.float32)
        d_ex.sort(axis=1)
        sm4_all[b * MS:(b + 1) * MS] = d_ex[:, :TAIL]
    tails = sm4_all.mean(-1)
    far = np.argsort(-tails)[:FAR]
    return float((sm4_all[far].astype(np.float64) ** 2).mean())


def _size_loss_host(latents):
    norms = np.abs(latents).sum(-1, dtype=np.float64)
    viol = np.maximum(norms - 1.0, 0.0)
    return float((viol ** 2).mean())


def kernel(latent_states, latent_actions, state_space_samples,
           action_space_samples, _want_results=False, _trace=False):
    latent_states = np.asarray(latent_states, np.float32)
    latent_actions = np.asarray(latent_actions, np.float32)
    state_space_samples = np.asarray(state_space_samples, np.float32)
    action_space_samples = np.asarray(action_space_samples, np.float32)

    nc = _get_nc()
    in_maps, host = _make_in_maps(latent_states, latent_actions,
                                  state_space_samples, action_space_samples)
    res = run_bass_kernel_spmd(nc, in_maps, core_ids=list(range(8)),
                               trace=_trace)
    total = np.float64(0)
    total += _size_loss_host(latent_states)
    total += _size_loss_host(latent_actions)
    total += _cov_loss_host(res.results, host, "s", state_space_samples,
                            latent_states)
    total += _cov_loss_host(res.results, host, "a", action_space_samples,
                            latent_actions)
    out = np.float32(total)
    if _want_results:
        return out, res
    return out


# revision 25
# speedup vs baseline: 3.4979x; 1.1624x over previous
"""CoverageLoss kernel for 8 Trainium2 NeuronCores — "ship-all" design.

Math: loss = size(ls) + size(la) + cov(ss, ls) + cov(sa, la)
  cov(S, L): d = cdist_l1(S, L); sm4 = 4 smallest per row; tail = sm4.mean(-1)
             far = top64(tail); loss = mean(sm4[far]**2)

Device strategy (4 latent shards x 2 sample shards = 8 cores):
  One-sided thermometer quantization: latents are snapped to a Q=8 Lloyd
  ladder (per-coordinate, Gaussian), samples stay EXACT.  For sample value s
  and quantized latent c_j, |s - c_j| is linear in the thermometer bits
  g_q = sign(l - t_q), so a single fp8 DoubleRow matmul
  M[s, l] = <u(s), g(l)> gives d_q(s, l) = A(s) + M[s, l] = sum_e |s_e - c(l_e)|
  exactly (up to fp8 rounding of u).  The device ships the entire quantized
  distance matrix back (uint8 with per-sample affine on the ACT engine /
  fp16 on DVE); the host does top-24 candidate selection, exact fp32 L1
  refinement, top-64 far selection, and the final scalar.  Size losses are
  host-side (trivially small).
"""

import numpy as np
import ml_dtypes
from contextlib import ExitStack

import concourse.bass as bass
import concourse.bacc as bacc
import concourse.mybir as mybir
import concourse.tile as tile
from concourse.bass_utils import run_bass_kernel_spmd

# ---- problem constants ----
NLAT, ES, EA = 8192, 64, 32
NSMP = 2048
TAIL, FAR = 4, 64
A_SHARDS, B_SHARDS = 4, 2
NL = NLAT // A_SHARDS              # 2048 latents per core
MS = NSMP // B_SHARDS              # 1024 samples per core
NTILES = MS // 128                 # 8 sample tiles

Q_S = 2                            # thermometer levels per state coordinate
Q_A = 4                            # thermometer levels per action coordinate
KSUB_S = 1                         # plain 128-deep fp8 chunks
KSUB_A = 1
KC_S = ES * Q_S // (128 * KSUB_S)  # 1 chunk (states)
KC_A = EA * Q_A // (128 * KSUB_A)  # 1 chunk (actions)
NCHUNK = NL // 512                 # 4 psum column chunks

NCAND = 128                        # host-side candidate count per sample
U8_SCALE = 3.0                     # uint8 ladder: out = 250 - 3*d
U8_BIAS0 = 250.0

F32 = mybir.dt.float32
F16 = mybir.dt.float16
FP8 = mybir.dt.float8e4
U8 = mybir.dt.uint8

# psum half-tile consumer assignment: alternate ACT/DVE per half, ACT
# takes one extra (ACT's copy is slightly cheaper).  Both emit
# uint8 = relu(P + bias) with PSUM P = -3*M (-3 folded into coefficients).
def _on_act(half_idx):
    return half_idx % 2 == 0 or half_idx == 31


# ---- quantizer (host) ----
def _gauss_quantizer(Q):
    """Thresholds at Gaussian quantiles; ladder = Lloyd centroids."""
    from scipy.stats import norm
    qs = (np.arange(Q) + 0.5) / Q
    t = norm.ppf(qs)
    edges = np.concatenate([[-np.inf], t, [np.inf]])
    a, b = edges[:-1], edges[1:]
    c = (norm.pdf(a) - norm.pdf(b)) / np.maximum(norm.cdf(b) - norm.cdf(a), 1e-12)
    # solve-map:  |s - c_j| = A + sum_q u_q G[j, q],  G[j,q] = +1 if q<j else -1
    G = np.where(np.arange(Q)[None, :] < np.arange(Q + 1)[:, None], 1.0, -1.0)
    M = np.concatenate([np.ones((Q + 1, 1)), G], 1)
    return t.astype(np.float64), c.astype(np.float64), np.linalg.inv(M)


_QZ = {q: _gauss_quantizer(q) for q in {Q_S, Q_A}}


def _encode_samples(S, Q):
    """[n, e] samples -> (A_sum [n], U [n, e, Q] fp32 coefficient tensor)."""
    t, c, minv = _QZ[Q]
    B = np.abs(S.astype(np.float64)[..., None] - c)         # [n, e, Q+1]
    X = B @ minv.T                                           # [n, e, Q+1]
    A = X[..., 0].sum(-1)                                    # [n]
    U = X[..., 1:]                                           # [n, e, Q]
    return A.astype(np.float32), U.astype(np.float32)


def _encode_latents(L, Q):
    """[N, e] latents -> g [N, e, Q] in {-1, +1}."""
    t, c, minv = _QZ[Q]
    return np.where(L[:, :, None] > t.astype(np.float32), 1.0, -1.0
                    ).astype(np.float32)


def _to_dr_layout(X, kc, ksub):
    """[rows, e, Q] -> [128, kc, ksub, rows] fp8 layout.

    flat contraction index f = e*Q + q maps to (kc, ksub, kpart):
    f = ((kc*ksub) + ks)*128 + kpart.
    """
    rows = X.shape[0]
    Xf = X.reshape(rows, -1).T                               # [e*Q, rows]
    Xf = Xf.reshape(kc, ksub, 128, rows).transpose(2, 0, 1, 3)
    return np.ascontiguousarray(Xf).astype(ml_dtypes.float8_e4m3)


# ---- device kernel ----
def _build_nc():
    nc = bacc.Bacc("TRN2", target_bir_lowering=False, debug=False,
                   num_devices=8)
    inp = {}
    for name, shape in [
        ("bl_s", [128, KC_S, KSUB_S, NL]), ("bs_s", [128, KC_S, KSUB_S, MS]),
        ("bl_a", [128, KC_A, KSUB_A, NL]), ("bs_a", [128, KC_A, KSUB_A, MS]),
    ]:
        inp[name] = nc.dram_tensor(name, shape, FP8, kind="ExternalInput").ap()
    for name in ("biasv_s", "biasv_a"):
        inp[name] = nc.dram_tensor(name, [128, NTILES], F32,
                                   kind="ExternalInput").ap()
    inp["warmT"] = nc.dram_tensor("warmT", [128, 512], FP8,
                                  kind="ExternalInput").ap()
    qd8 = nc.dram_tensor("qd8", [128, NTILES, 2, NL], U8,
                         kind="ExternalOutput").ap()

    with tile.TileContext(nc) as tc:
        with ExitStack() as ctx:
            big = ctx.enter_context(tc.tile_pool(name="bigin", bufs=1))
            psum = ctx.enter_context(tc.tile_pool(name="psum", bufs=4,
                                                  space="PSUM"))
            st8 = ctx.enter_context(tc.tile_pool(name="st8", bufs=4))

            # warm-up weights first: tiny DMA, PE dummies run during the
            # big input DMAs so HAM is at 8/8 when real matmuls start
            warmt = big.tile([128, 512], FP8, tag="warmT")
            nc.sync.dma_start(warmt[:], inp["warmT"][:, :])
            psw = psum.tile([128, NL // 2], F32, tag="ps")
            for _ in range(10):
                nc.tensor.matmul(
                    psw[:, 0:512], lhsT=warmt[:, 0:128], rhs=warmt[:, 0:512],
                    start=True, stop=True, skip_group_check=True)

            # biases (small, needed by every consumer op)
            bias = {}
            for name in ("biasv_s", "biasv_a"):
                t = big.tile([128, NTILES], F32, tag=name)
                nc.sync.dma_start(t[:], inp[name][:, :])
                bias[name] = t
            # negated biases for the DVE path: out = (P - (-bias)) max 0
            nbias = {}
            for name in ("biasv_s", "biasv_a"):
                t = big.tile([128, NTILES], F32, tag=f"n{name}")
                nc.vector.tensor_scalar(
                    out=t[:], in0=bias[name][:], scalar1=-1.0, scalar2=None,
                    op0=mybir.AluOpType.mult)
                nbias[name] = t
            # encoded inputs, split across DMA issue engines
            enc = {}
            for name, eng in (("bs_s", nc.sync), ("bl_s", nc.sync),
                              ("bs_a", nc.gpsimd), ("bl_a", nc.gpsimd)):
                t = big.tile(list(inp[name].shape), FP8, tag=name)
                eng.dma_start(t[:, 0, :, :], inp[name][:, 0, :, :])
                enc[name] = t

            # ACT table pre-warm (Relu) while DMAs stream
            warm8 = st8.tile([128, 2, NL], U8, tag="qt8")
            nc.scalar.activation(
                warm8[:, 0, 0:8], bias["biasv_s"][:, 0:8],
                mybir.ActivationFunctionType.Relu, bias=0.0, scale=1.0)

            covs = {
                "s": (enc["bl_s"], enc["bs_s"], bias["biasv_s"],
                      nbias["biasv_s"]),
                "a": (enc["bl_a"], enc["bs_a"], bias["biasv_a"],
                      nbias["biasv_a"]),
            }
            half_idx = 0
            for m in range(NTILES):
                qt = st8.tile([128, 2, NL], U8, tag="qt8")
                for ci, cov in enumerate(("s", "a")):
                    bl, bs, bv, nbv = covs[cov]
                    lhsT = bs[:, 0, 0, m * 128:(m + 1) * 128]
                    for h in range(2):
                        ps = psum.tile([128, NL // 2], F32, tag="ps")
                        for n in range(2):
                            j = h * 2 + n
                            nc.tensor.matmul(
                                ps[:, n * 512:(n + 1) * 512],
                                lhsT=lhsT,
                                rhs=bl[:, 0, 0, j * 512:(j + 1) * 512],
                                start=True, stop=True,
                                skip_group_check=True)
                        qslice = qt[:, ci, h * (NL // 2):(h + 1) * (NL // 2)]
                        if _on_act(half_idx):
                            nc.scalar.activation(
                                qslice, ps[:],
                                mybir.ActivationFunctionType.Relu,
                                bias=bv[:, m:m + 1], scale=1.0)
                        else:
                            nc.vector.tensor_scalar(
                                out=qslice, in0=ps[:],
                                scalar1=nbv[:, m:m + 1],
                                scalar2=0.0, op0=mybir.AluOpType.subtract,
                                op1=mybir.AluOpType.max)
                        half_idx += 1
                eng = nc.sync if m % 2 == 0 else nc.gpsimd
                eng.dma_start(qd8[:, m, :, :], qt[:])
    nc.compile()
    return nc


_NC_CACHE = {}


def _get_nc():
    if "nc" not in _NC_CACHE:
        _NC_CACHE["nc"] = _build_nc()
    return _NC_CACHE["nc"]


# ---- host pre/post ----
def _make_in_maps(latent_states, latent_actions, state_space_samples,
                  action_space_samples):
    g_s = _encode_latents(latent_states, Q_S)   # [8192, 64, Q_S]
    g_a = _encode_latents(latent_actions, Q_A)  # [8192, 32, Q_A]
    A_s, U_s = _encode_samples(state_space_samples, Q_S)
    A_a, U_a = _encode_samples(action_space_samples, Q_A)
    warm = np.full((128, 512), 0.25, ml_dtypes.float8_e4m3)

    in_maps = []
    host = []                                  # per-core host context
    for core in range(8):
        a, b = core % A_SHARDS, core // A_SHARDS
        sl_l = slice(a * NL, (a + 1) * NL)
        sl_m = slice(b * MS, (b + 1) * MS)
        A_sb = A_s[sl_m]
        A_ab = A_a[sl_m]
        bias_s = (U8_BIAS0 - U8_SCALE *
                  A_sb.reshape(NTILES, 128).T).astype(np.float32)
        bias_a = (U8_BIAS0 - U8_SCALE *
                  A_ab.reshape(NTILES, 128).T).astype(np.float32)
        # -U8_SCALE folded into the sample coefficients: PSUM P = -3*M
        in_maps.append({
            "bl_s": _to_dr_layout(g_s[sl_l], KC_S, KSUB_S),
            "bs_s": _to_dr_layout(-U8_SCALE * U_s[sl_m], KC_S, KSUB_S),
            "bl_a": _to_dr_layout(g_a[sl_l], KC_A, KSUB_A),
            "bs_a": _to_dr_layout(-U8_SCALE * U_a[sl_m], KC_A, KSUB_A),
            "biasv_s": np.ascontiguousarray(bias_s),
            "biasv_a": np.ascontiguousarray(bias_a),
            "warmT": warm,
        })
        host.append({"a": a, "b": b})
    return in_maps, host


def _cov_loss_host(results, host, cov, samples, latents):
    """Assemble quantized rankings, exact-refine top candidates, compute
    the coverage loss term."""
    ci = 0 if cov == "s" else 1
    sm4_all = np.empty((NSMP, TAIL), np.float32)
    for b in range(B_SHARDS):
        cores = [b * A_SHARDS + a for a in range(A_SHARDS)]
        # rank score: larger = closer (uint8 = relu(250 - 3*d))
        score = np.empty((MS, A_SHARDS * NL), np.uint8)
        for a, c in enumerate(cores):
            r8 = results[c]["qd8"]    # [128, NTILES, 2, NL] uint8
            score[:, a * NL:(a + 1) * NL] = \
                r8[:, :, ci, :].transpose(1, 0, 2).reshape(MS, NL)
        idx = np.argpartition(-score.astype(np.int16), NCAND,
                              axis=1)[:, :NCAND]
        smp = samples[b * MS:(b + 1) * MS]
        cand = latents[idx]                          # [MS, NCAND, e]
        d_ex = np.abs(smp[:, None, :] - cand).sum(-1, dtype=np.float32)
        d_ex.sort(axis=1)
        sm4_all[b * MS:(b + 1) * MS] = d_ex[:, :TAIL]
    tails = sm4_all.mean(-1)
    far = np.argsort(-tails)[:FAR]
    return float((sm4_all[far].astype(np.float64) ** 2).mean())


def _size_loss_host(latents):
    norms = np.abs(latents).sum(-1, dtype=np.float64)
    viol = np.maximum(norms - 1.0, 0.0)
    return float((viol ** 2).mean())


def kernel(latent_states, latent_actions, state_space_samples,
           action_space_samples, _want_results=False, _trace=False):
    latent_states = np.asarray(latent_states, np.float32)
    latent_actions = np.asarray(latent_actions, np.float32)
    state_space_samples = np.asarray(state_space_samples, np.float32)
    action_space_samples = np.asarray(action_space_samples, np.float32)

    nc = _get_nc()
    in_maps, host = _make_in_maps(latent_states, latent_actions,
                                  state_space_samples, action_space_samples)
    res = run_bass_kernel_spmd(nc, in_maps, core_ids=list(range(8)),
                               trace=_trace)
    total = np.float64(0)
    total += _size_loss_host(latent_states)
    total += _size_loss_host(latent_actions)
    total += _cov_loss_host(res.results, host, "s", state_space_samples,
                            latent_states)
    total += _cov_loss_host(res.results, host, "a", action_space_samples,
                            latent_actions)
    out = np.float32(total)
    if _want_results:
        return out, res
    return out


# revision 26
# speedup vs baseline: 3.5151x; 1.0049x over previous
"""CoverageLoss kernel for 8 Trainium2 NeuronCores — "ship-all" design.

Math: loss = size(ls) + size(la) + cov(ss, ls) + cov(sa, la)
  cov(S, L): d = cdist_l1(S, L); sm4 = 4 smallest per row; tail = sm4.mean(-1)
             far = top64(tail); loss = mean(sm4[far]**2)

Device strategy (4 latent shards x 2 sample shards = 8 cores):
  One-sided thermometer quantization: latents are snapped to a Q=8 Lloyd
  ladder (per-coordinate, Gaussian), samples stay EXACT.  For sample value s
  and quantized latent c_j, |s - c_j| is linear in the thermometer bits
  g_q = sign(l - t_q), so a single fp8 DoubleRow matmul
  M[s, l] = <u(s), g(l)> gives d_q(s, l) = A(s) + M[s, l] = sum_e |s_e - c(l_e)|
  exactly (up to fp8 rounding of u).  The device ships the entire quantized
  distance matrix back (uint8 with per-sample affine on the ACT engine /
  fp16 on DVE); the host does top-24 candidate selection, exact fp32 L1
  refinement, top-64 far selection, and the final scalar.  Size losses are
  host-side (trivially small).
"""

import numpy as np
import ml_dtypes
from contextlib import ExitStack

import concourse.bass as bass
import concourse.bacc as bacc
import concourse.mybir as mybir
import concourse.tile as tile
from concourse.bass_utils import run_bass_kernel_spmd

# ---- problem constants ----
NLAT, ES, EA = 8192, 64, 32
NSMP = 2048
TAIL, FAR = 4, 64
A_SHARDS, B_SHARDS = 4, 2
NL = NLAT // A_SHARDS              # 2048 latents per core
MS = NSMP // B_SHARDS              # 1024 samples per core
NTILES = MS // 128                 # 8 sample tiles

Q_S = 2                            # thermometer levels per state coordinate
Q_A = 4                            # thermometer levels per action coordinate
KSUB_S = 1                         # plain 128-deep fp8 chunks
KSUB_A = 1
KC_S = ES * Q_S // (128 * KSUB_S)  # 1 chunk (states)
KC_A = EA * Q_A // (128 * KSUB_A)  # 1 chunk (actions)
NCHUNK = NL // 512                 # 4 psum column chunks

NCAND = 128                        # host-side candidate count per sample
U8_SCALE = 3.0                     # uint8 ladder: out = 250 - 3*d
U8_BIAS0 = 250.0

F32 = mybir.dt.float32
F16 = mybir.dt.float16
FP8 = mybir.dt.float8e4
U8 = mybir.dt.uint8

# psum half-tile consumer assignment: alternate ACT/DVE per half, ACT
# takes one extra (ACT's copy is slightly cheaper).  Both emit
# uint8 = relu(P + bias) with PSUM P = -3*M (-3 folded into coefficients).
def _on_act(half_idx):
    return half_idx % 2 == 0 or half_idx == 31


# ---- quantizer (host) ----
def _gauss_quantizer(Q):
    """Thresholds at Gaussian quantiles; ladder = Lloyd centroids."""
    from scipy.stats import norm
    qs = (np.arange(Q) + 0.5) / Q
    t = norm.ppf(qs)
    edges = np.concatenate([[-np.inf], t, [np.inf]])
    a, b = edges[:-1], edges[1:]
    c = (norm.pdf(a) - norm.pdf(b)) / np.maximum(norm.cdf(b) - norm.cdf(a), 1e-12)
    # solve-map:  |s - c_j| = A + sum_q u_q G[j, q],  G[j,q] = +1 if q<j else -1
    G = np.where(np.arange(Q)[None, :] < np.arange(Q + 1)[:, None], 1.0, -1.0)
    M = np.concatenate([np.ones((Q + 1, 1)), G], 1)
    return t.astype(np.float64), c.astype(np.float64), np.linalg.inv(M)


_QZ = {q: _gauss_quantizer(q) for q in {Q_S, Q_A}}


def _encode_samples(S, Q):
    """[n, e] samples -> (A_sum [n], U [n, e, Q] fp32 coefficient tensor)."""
    t, c, minv = _QZ[Q]
    B = np.abs(S.astype(np.float64)[..., None] - c)         # [n, e, Q+1]
    X = B @ minv.T                                           # [n, e, Q+1]
    A = X[..., 0].sum(-1)                                    # [n]
    U = X[..., 1:]                                           # [n, e, Q]
    return A.astype(np.float32), U.astype(np.float32)


def _encode_latents(L, Q):
    """[N, e] latents -> g [N, e, Q] in {-1, +1}."""
    t, c, minv = _QZ[Q]
    return np.where(L[:, :, None] > t.astype(np.float32), 1.0, -1.0
                    ).astype(np.float32)


def _to_dr_layout(X, kc, ksub):
    """[rows, e, Q] -> [128, kc, ksub, rows] fp8 layout.

    flat contraction index f = e*Q + q maps to (kc, ksub, kpart):
    f = ((kc*ksub) + ks)*128 + kpart.
    """
    rows = X.shape[0]
    Xf = X.reshape(rows, -1).T                               # [e*Q, rows]
    Xf = Xf.reshape(kc, ksub, 128, rows).transpose(2, 0, 1, 3)
    return np.ascontiguousarray(Xf).astype(ml_dtypes.float8_e4m3)


# ---- device kernel ----
def _build_nc():
    nc = bacc.Bacc("TRN2", target_bir_lowering=False, debug=False,
                   num_devices=8)
    inp = {}
    for name, shape in [
        ("bl_s", [128, KC_S, KSUB_S, NL]), ("bs_s", [128, KC_S, KSUB_S, MS]),
        ("bl_a", [128, KC_A, KSUB_A, NL]), ("bs_a", [128, KC_A, KSUB_A, MS]),
    ]:
        inp[name] = nc.dram_tensor(name, shape, FP8, kind="ExternalInput").ap()
    for name in ("biasv_s", "biasv_a"):
        inp[name] = nc.dram_tensor(name, [128, NTILES], F32,
                                   kind="ExternalInput").ap()
    inp["warmT"] = nc.dram_tensor("warmT", [128, 512], FP8,
                                  kind="ExternalInput").ap()
    qd8 = nc.dram_tensor("qd8", [128, NTILES, 2, NL], U8,
                         kind="ExternalOutput").ap()

    with tile.TileContext(nc) as tc:
        with ExitStack() as ctx:
            big = ctx.enter_context(tc.tile_pool(name="bigin", bufs=1))
            psum = ctx.enter_context(tc.tile_pool(name="psum", bufs=4,
                                                  space="PSUM"))
            st8 = ctx.enter_context(tc.tile_pool(name="st8", bufs=4))

            # warm-up weights first: tiny DMA, PE dummies run during the
            # big input DMAs so HAM is at 8/8 when real matmuls start
            warmt = big.tile([128, 512], FP8, tag="warmT")
            nc.sync.dma_start(warmt[:], inp["warmT"][:, :])
            psw = psum.tile([128, NL // 2], F32, tag="ps")
            for _ in range(4):
                nc.tensor.matmul(
                    psw[:, 0:512], lhsT=warmt[:, 0:128], rhs=warmt[:, 0:512],
                    start=True, stop=True, skip_group_check=True)

            # biases (small, needed by every consumer op)
            bias = {}
            for name in ("biasv_s", "biasv_a"):
                t = big.tile([128, NTILES], F32, tag=name)
                nc.sync.dma_start(t[:], inp[name][:, :])
                bias[name] = t
            # negated biases for the DVE path: out = (P - (-bias)) max 0
            nbias = {}
            for name in ("biasv_s", "biasv_a"):
                t = big.tile([128, NTILES], F32, tag=f"n{name}")
                nc.vector.tensor_scalar(
                    out=t[:], in0=bias[name][:], scalar1=-1.0, scalar2=None,
                    op0=mybir.AluOpType.mult)
                nbias[name] = t
            # encoded inputs, split across DMA issue engines
            enc = {}
            for name, eng in (("bs_s", nc.sync), ("bl_s", nc.sync),
                              ("bs_a", nc.gpsimd), ("bl_a", nc.gpsimd)):
                t = big.tile(list(inp[name].shape), FP8, tag=name)
                eng.dma_start(t[:, 0, :, :], inp[name][:, 0, :, :])
                enc[name] = t

            # ACT table pre-warm (Relu) while DMAs stream
            warm8 = st8.tile([128, 2, NL], U8, tag="qt8")
            nc.scalar.activation(
                warm8[:, 0, 0:8], bias["biasv_s"][:, 0:8],
                mybir.ActivationFunctionType.Relu, bias=0.0, scale=1.0)

            covs = {
                "s": (enc["bl_s"], enc["bs_s"], bias["biasv_s"],
                      nbias["biasv_s"]),
                "a": (enc["bl_a"], enc["bs_a"], bias["biasv_a"],
                      nbias["biasv_a"]),
            }
            half_idx = 0
            for m in range(NTILES):
                qt = st8.tile([128, 2, NL], U8, tag="qt8")
                for ci, cov in enumerate(("s", "a")):
                    bl, bs, bv, nbv = covs[cov]
                    lhsT = bs[:, 0, 0, m * 128:(m + 1) * 128]
                    for h in range(2):
                        ps = psum.tile([128, NL // 2], F32, tag="ps")
                        for n in range(2):
                            j = h * 2 + n
                            nc.tensor.matmul(
                                ps[:, n * 512:(n + 1) * 512],
                                lhsT=lhsT,
                                rhs=bl[:, 0, 0, j * 512:(j + 1) * 512],
                                start=True, stop=True,
                                skip_group_check=True)
                        qslice = qt[:, ci, h * (NL // 2):(h + 1) * (NL // 2)]
                        if _on_act(half_idx):
                            nc.scalar.activation(
                                qslice, ps[:],
                                mybir.ActivationFunctionType.Relu,
                                bias=bv[:, m:m + 1], scale=1.0)
                        else:
                            nc.vector.tensor_scalar(
                                out=qslice, in0=ps[:],
                                scalar1=nbv[:, m:m + 1],
                                scalar2=0.0, op0=mybir.AluOpType.subtract,
                                op1=mybir.AluOpType.max)
                        half_idx += 1
                eng = nc.sync if m % 2 == 0 else nc.gpsimd
                eng.dma_start(qd8[:, m, :, :], qt[:])
    nc.compile()
    return nc


_NC_CACHE = {}


def _get_nc():
    if "nc" not in _NC_CACHE:
        _NC_CACHE["nc"] = _build_nc()
    return _NC_CACHE["nc"]


# ---- host pre/post ----
def _make_in_maps(latent_states, latent_actions, state_space_samples,
                  action_space_samples):
    g_s = _encode_latents(latent_states, Q_S)   # [8192, 64, Q_S]
    g_a = _encode_latents(latent_actions, Q_A)  # [8192, 32, Q_A]
    A_s, U_s = _encode_samples(state_space_samples, Q_S)
    A_a, U_a = _encode_samples(action_space_samples, Q_A)
    warm = np.full((128, 512), 0.25, ml_dtypes.float8_e4m3)

    in_maps = []
    host = []                                  # per-core host context
    for core in range(8):
        a, b = core % A_SHARDS, core // A_SHARDS
        sl_l = slice(a * NL, (a + 1) * NL)
        sl_m = slice(b * MS, (b + 1) * MS)
        A_sb = A_s[sl_m]
        A_ab = A_a[sl_m]
        bias_s = (U8_BIAS0 - U8_SCALE *
                  A_sb.reshape(NTILES, 128).T).astype(np.float32)
        bias_a = (U8_BIAS0 - U8_SCALE *
                  A_ab.reshape(NTILES, 128).T).astype(np.float32)
        # -U8_SCALE folded into the sample coefficients: PSUM P = -3*M
        in_maps.append({
            "bl_s": _to_dr_layout(g_s[sl_l], KC_S, KSUB_S),
            "bs_s": _to_dr_layout(-U8_SCALE * U_s[sl_m], KC_S, KSUB_S),
            "bl_a": _to_dr_layout(g_a[sl_l], KC_A, KSUB_A),
            "bs_a": _to_dr_layout(-U8_SCALE * U_a[sl_m], KC_A, KSUB_A),
            "biasv_s": np.ascontiguousarray(bias_s),
            "biasv_a": np.ascontiguousarray(bias_a),
            "warmT": warm,
        })
        host.append({"a": a, "b": b})
    return in_maps, host


def _cov_loss_host(results, host, cov, samples, latents):
    """Assemble quantized rankings, exact-refine top candidates, compute
    the coverage loss term."""
    ci = 0 if cov == "s" else 1
    sm4_all = np.empty((NSMP, TAIL), np.float32)
    for b in range(B_SHARDS):
        cores = [b * A_SHARDS + a for a in range(A_SHARDS)]
        # rank score: larger = closer (uint8 = relu(250 - 3*d))
        score = np.empty((MS, A_SHARDS * NL), np.uint8)
        for a, c in enumerate(cores):
            r8 = results[c]["qd8"]    # [128, NTILES, 2, NL] uint8
            score[:, a * NL:(a + 1) * NL] = \
                r8[:, :, ci, :].transpose(1, 0, 2).reshape(MS, NL)
        idx = np.argpartition(-score.astype(np.int16), NCAND,
                              axis=1)[:, :NCAND]
        smp = samples[b * MS:(b + 1) * MS]
        cand = latents[idx]                          # [MS, NCAND, e]
        d_ex = np.abs(smp[:, None, :] - cand).sum(-1, dtype=np.float32)
        d_ex.sort(axis=1)
        sm4_all[b * MS:(b + 1) * MS] = d_ex[:, :TAIL]
    tails = sm4_all.mean(-1)
    far = np.argsort(-tails)[:FAR]
    return float((sm4_all[far].astype(np.float64) ** 2).mean())


def _size_loss_host(latents):
    norms = np.abs(latents).sum(-1, dtype=np.float64)
    viol = np.maximum(norms - 1.0, 0.0)
    return float((viol ** 2).mean())


def kernel(latent_states, latent_actions, state_space_samples,
           action_space_samples, _want_results=False, _trace=False):
    latent_states = np.asarray(latent_states, np.float32)
    latent_actions = np.asarray(latent_actions, np.float32)
    state_space_samples = np.asarray(state_space_samples, np.float32)
    action_space_samples = np.asarray(action_space_samples, np.float32)

    nc = _get_nc()
    in_maps, host = _make_in_maps(latent_states, latent_actions,
                                  state_space_samples, action_space_samples)
    res = run_bass_kernel_spmd(nc, in_maps, core_ids=list(range(8)),
                               trace=_trace)
    total = np.float64(0)
    total += _size_loss_host(latent_states)
    total += _size_loss_host(latent_actions)
    total += _cov_loss_host(res.results, host, "s", state_space_samples,
                            latent_states)
    total += _cov_loss_host(res.results, host, "a", action_space_samples,
                            latent_actions)
    out = np.float32(total)
    if _want_results:
        return out, res
    return out


# revision 28
# speedup vs baseline: 3.7210x; 1.0586x over previous
"""CoverageLoss kernel for 8 Trainium2 NeuronCores — "ship-all" design.

Math: loss = size(ls) + size(la) + cov(ss, ls) + cov(sa, la)
  cov(S, L): d = cdist_l1(S, L); sm4 = 4 smallest per row; tail = sm4.mean(-1)
             far = top64(tail); loss = mean(sm4[far]**2)

Device strategy (4 latent shards x 2 sample shards = 8 cores):
  One-sided thermometer quantization: latents are snapped to a Q=8 Lloyd
  ladder (per-coordinate, Gaussian), samples stay EXACT.  For sample value s
  and quantized latent c_j, |s - c_j| is linear in the thermometer bits
  g_q = sign(l - t_q), so a single fp8 DoubleRow matmul
  M[s, l] = <u(s), g(l)> gives d_q(s, l) = A(s) + M[s, l] = sum_e |s_e - c(l_e)|
  exactly (up to fp8 rounding of u).  The device ships the entire quantized
  distance matrix back (uint8 with per-sample affine on the ACT engine /
  fp16 on DVE); the host does top-24 candidate selection, exact fp32 L1
  refinement, top-64 far selection, and the final scalar.  Size losses are
  host-side (trivially small).
"""

import numpy as np
import ml_dtypes
from contextlib import ExitStack

import concourse.bass as bass
import concourse.bacc as bacc
import concourse.mybir as mybir
import concourse.tile as tile
from concourse.bass_utils import run_bass_kernel_spmd

# ---- problem constants ----
NLAT, ES, EA = 8192, 64, 32
NSMP = 2048
TAIL, FAR = 4, 64
A_SHARDS, B_SHARDS = 4, 2
NL = NLAT // A_SHARDS              # 2048 latents per core
MS = NSMP // B_SHARDS              # 1024 samples per core
NTILES = MS // 128                 # 8 sample tiles

Q_S = 2                            # thermometer levels per state coordinate
Q_A = 4                            # thermometer levels per action coordinate
KSUB_S = 1                         # plain 128-deep fp8 chunks
KSUB_A = 1
KC_S = ES * Q_S // (128 * KSUB_S)  # 1 chunk (states)
KC_A = EA * Q_A // (128 * KSUB_A)  # 1 chunk (actions)
NCHUNK = NL // 512                 # 4 psum column chunks

NCAND = 128                        # host-side candidate count per sample
U8_SCALE = 3.0                     # uint8 ladder: out = 250 - 3*d
U8_BIAS0 = 250.0

F32 = mybir.dt.float32
F16 = mybir.dt.float16
FP8 = mybir.dt.float8e4
U8 = mybir.dt.uint8

# psum half-tile consumer assignment: alternate ACT/DVE per half, ACT
# takes one extra (ACT's copy is slightly cheaper).  Both emit
# uint8 = relu(P + bias) with PSUM P = -3*M (-3 folded into coefficients).
def _on_act(half_idx):
    return half_idx % 2 == 0 or half_idx == 31


# ---- quantizer (host) ----
def _gauss_quantizer(Q):
    """Thresholds at Gaussian quantiles; ladder = Lloyd centroids."""
    from scipy.stats import norm
    qs = (np.arange(Q) + 0.5) / Q
    t = norm.ppf(qs)
    edges = np.concatenate([[-np.inf], t, [np.inf]])
    a, b = edges[:-1], edges[1:]
    c = (norm.pdf(a) - norm.pdf(b)) / np.maximum(norm.cdf(b) - norm.cdf(a), 1e-12)
    # solve-map:  |s - c_j| = A + sum_q u_q G[j, q],  G[j,q] = +1 if q<j else -1
    G = np.where(np.arange(Q)[None, :] < np.arange(Q + 1)[:, None], 1.0, -1.0)
    M = np.concatenate([np.ones((Q + 1, 1)), G], 1)
    return t.astype(np.float64), c.astype(np.float64), np.linalg.inv(M)


_QZ = {q: _gauss_quantizer(q) for q in {Q_S, Q_A}}


def _encode_samples(S, Q):
    """[n, e] samples -> (A_sum [n], U [n, e, Q] fp32 coefficient tensor)."""
    t, c, minv = _QZ[Q]
    B = np.abs(S.astype(np.float64)[..., None] - c)         # [n, e, Q+1]
    X = B @ minv.T                                           # [n, e, Q+1]
    A = X[..., 0].sum(-1)                                    # [n]
    U = X[..., 1:]                                           # [n, e, Q]
    return A.astype(np.float32), U.astype(np.float32)


def _encode_latents(L, Q):
    """[N, e] latents -> g [N, e, Q] in {-1, +1}."""
    t, c, minv = _QZ[Q]
    return np.where(L[:, :, None] > t.astype(np.float32), 1.0, -1.0
                    ).astype(np.float32)


def _to_dr_layout(X, kc, ksub):
    """[rows, e, Q] -> [128, kc, ksub, rows] fp8 layout.

    flat contraction index f = e*Q + q maps to (kc, ksub, kpart):
    f = ((kc*ksub) + ks)*128 + kpart.
    """
    rows = X.shape[0]
    Xf = X.reshape(rows, -1).T                               # [e*Q, rows]
    Xf = Xf.reshape(kc, ksub, 128, rows).transpose(2, 0, 1, 3)
    return np.ascontiguousarray(Xf).astype(ml_dtypes.float8_e4m3)


# ---- device kernel ----
def _build_nc():
    nc = bacc.Bacc("TRN2", target_bir_lowering=False, debug=False,
                   num_devices=8)
    inp = {}
    for name, shape in [
        ("bl_s", [128, KC_S, KSUB_S, NL]), ("bs_s", [128, KC_S, KSUB_S, MS]),
        ("bl_a", [128, KC_A, KSUB_A, NL]), ("bs_a", [128, KC_A, KSUB_A, MS]),
    ]:
        inp[name] = nc.dram_tensor(name, shape, FP8, kind="ExternalInput").ap()
    for name in ("biasv_s", "biasv_a"):
        inp[name] = nc.dram_tensor(name, [128, NTILES], F32,
                                   kind="ExternalInput").ap()
    inp["warmT"] = nc.dram_tensor("warmT", [128, 512], FP8,
                                  kind="ExternalInput").ap()
    qd8 = nc.dram_tensor("qd8", [128, NTILES, 2, NL], U8,
                         kind="ExternalOutput").ap()

    with tile.TileContext(nc) as tc:
        with ExitStack() as ctx:
            big = ctx.enter_context(tc.tile_pool(name="bigin", bufs=1))
            psum = ctx.enter_context(tc.tile_pool(name="psum", bufs=4,
                                                  space="PSUM"))
            st8 = ctx.enter_context(tc.tile_pool(name="st8", bufs=4))

            # warm-up weights first: tiny DMA, PE dummies run during the
            # big input DMAs so HAM is at 8/8 when real matmuls start
            warmt = big.tile([128, 512], FP8, tag="warmT")
            nc.sync.dma_start(warmt[:], inp["warmT"][:, :])
            psw = psum.tile([128, NL // 2], F32, tag="ps")
            for _ in range(4):
                nc.tensor.matmul(
                    psw[:, 0:512], lhsT=warmt[:, 0:128], rhs=warmt[:, 0:512],
                    start=True, stop=True, skip_group_check=True)

            # biases (small, needed by every consumer op)
            bias = {}
            for name in ("biasv_s", "biasv_a"):
                t = big.tile([128, NTILES], F32, tag=name)
                nc.sync.dma_start(t[:], inp[name][:, :])
                bias[name] = t
            # negated biases for the DVE path: out = (P - (-bias)) max 0
            nbias = {}
            for name in ("biasv_s", "biasv_a"):
                t = big.tile([128, NTILES], F32, tag=f"n{name}")
                nc.vector.tensor_scalar(
                    out=t[:], in0=bias[name][:], scalar1=-1.0, scalar2=None,
                    op0=mybir.AluOpType.mult)
                nbias[name] = t
            # encoded inputs: spread across queue engines and column-chunk
            # the big tables so the first matmuls' data lands early
            enc = {}
            for name, eng, nch in (("bs_s", nc.sync, 2),
                                   ("bl_s", nc.scalar, 4),
                                   ("bs_a", nc.gpsimd, 2),
                                   ("bl_a", nc.gpsimd, 4)):
                t = big.tile(list(inp[name].shape), FP8, tag=name)
                w = inp[name].shape[-1] // nch
                for j in range(nch):
                    eng.dma_start(t[:, 0, :, j * w:(j + 1) * w],
                                  inp[name][:, 0, :, j * w:(j + 1) * w])
                enc[name] = t

            # ACT table pre-warm (Relu) while DMAs stream
            warm8 = st8.tile([128, 2, NL], U8, tag="qt8")
            nc.scalar.activation(
                warm8[:, 0, 0:8], bias["biasv_s"][:, 0:8],
                mybir.ActivationFunctionType.Relu, bias=0.0, scale=1.0)

            covs = {
                "s": (enc["bl_s"], enc["bs_s"], bias["biasv_s"],
                      nbias["biasv_s"]),
                "a": (enc["bl_a"], enc["bs_a"], bias["biasv_a"],
                      nbias["biasv_a"]),
            }
            half_idx = 0
            for m in range(NTILES):
                qt = st8.tile([128, 2, NL], U8, tag="qt8")
                for ci, cov in enumerate(("s", "a")):
                    bl, bs, bv, nbv = covs[cov]
                    lhsT = bs[:, 0, 0, m * 128:(m + 1) * 128]
                    for h in range(2):
                        ps = psum.tile([128, NL // 2], F32, tag="ps")
                        for n in range(2):
                            j = h * 2 + n
                            nc.tensor.matmul(
                                ps[:, n * 512:(n + 1) * 512],
                                lhsT=lhsT,
                                rhs=bl[:, 0, 0, j * 512:(j + 1) * 512],
                                start=True, stop=True,
                                skip_group_check=True)
                        qslice = qt[:, ci, h * (NL // 2):(h + 1) * (NL // 2)]
                        if _on_act(half_idx):
                            nc.scalar.activation(
                                qslice, ps[:],
                                mybir.ActivationFunctionType.Relu,
                                bias=bv[:, m:m + 1], scale=1.0)
                        else:
                            nc.vector.tensor_scalar(
                                out=qslice, in0=ps[:],
                                scalar1=nbv[:, m:m + 1],
                                scalar2=0.0, op0=mybir.AluOpType.subtract,
                                op1=mybir.AluOpType.max)
                        half_idx += 1
                eng = nc.sync if m % 2 == 0 else nc.gpsimd
                eng.dma_start(qd8[:, m, :, :], qt[:])
    nc.compile()
    return nc


_NC_CACHE = {}


def _get_nc():
    if "nc" not in _NC_CACHE:
        _NC_CACHE["nc"] = _build_nc()
    return _NC_CACHE["nc"]


# ---- host pre/post ----
def _make_in_maps(latent_states, latent_actions, state_space_samples,
                  action_space_samples):
    g_s = _encode_latents(latent_states, Q_S)   # [8192, 64, Q_S]
    g_a = _encode_latents(latent_actions, Q_A)  # [8192, 32, Q_A]
    A_s, U_s = _encode_samples(state_space_samples, Q_S)
    A_a, U_a = _encode_samples(action_space_samples, Q_A)
    warm = np.full((128, 512), 0.25, ml_dtypes.float8_e4m3)

    in_maps = []
    host = []                                  # per-core host context
    for core in range(8):
        a, b = core % A_SHARDS, core // A_SHARDS
        sl_l = slice(a * NL, (a + 1) * NL)
        sl_m = slice(b * MS, (b + 1) * MS)
        A_sb = A_s[sl_m]
        A_ab = A_a[sl_m]
        bias_s = (U8_BIAS0 - U8_SCALE *
                  A_sb.reshape(NTILES, 128).T).astype(np.float32)
        bias_a = (U8_BIAS0 - U8_SCALE *
                  A_ab.reshape(NTILES, 128).T).astype(np.float32)
        # -U8_SCALE folded into the sample coefficients: PSUM P = -3*M
        in_maps.append({
            "bl_s": _to_dr_layout(g_s[sl_l], KC_S, KSUB_S),
            "bs_s": _to_dr_layout(-U8_SCALE * U_s[sl_m], KC_S, KSUB_S),
            "bl_a": _to_dr_layout(g_a[sl_l], KC_A, KSUB_A),
            "bs_a": _to_dr_layout(-U8_SCALE * U_a[sl_m], KC_A, KSUB_A),
            "biasv_s": np.ascontiguousarray(bias_s),
            "biasv_a": np.ascontiguousarray(bias_a),
            "warmT": warm,
        })
        host.append({"a": a, "b": b})
    return in_maps, host


def _cov_loss_host(results, host, cov, samples, latents):
    """Assemble quantized rankings, exact-refine top candidates, compute
    the coverage loss term."""
    ci = 0 if cov == "s" else 1
    sm4_all = np.empty((NSMP, TAIL), np.float32)
    for b in range(B_SHARDS):
        cores = [b * A_SHARDS + a for a in range(A_SHARDS)]
        # rank score: larger = closer (uint8 = relu(250 - 3*d))
        score = np.empty((MS, A_SHARDS * NL), np.uint8)
        for a, c in enumerate(cores):
            r8 = results[c]["qd8"]    # [128, NTILES, 2, NL] uint8
            score[:, a * NL:(a + 1) * NL] = \
                r8[:, :, ci, :].transpose(1, 0, 2).reshape(MS, NL)
        idx = np.argpartition(-score.astype(np.int16), NCAND,
                              axis=1)[:, :NCAND]
        smp = samples[b * MS:(b + 1) * MS]
        cand = latents[idx]                          # [MS, NCAND, e]
        d_ex = np.abs(smp[:, None, :] - cand).sum(-1, dtype=np.float32)
        d_ex.sort(axis=1)
        sm4_all[b * MS:(b + 1) * MS] = d_ex[:, :TAIL]
    tails = sm4_all.mean(-1)
    far = np.argsort(-tails)[:FAR]
    return float((sm4_all[far].astype(np.float64) ** 2).mean())


def _size_loss_host(latents):
    norms = np.abs(latents).sum(-1, dtype=np.float64)
    viol = np.maximum(norms - 1.0, 0.0)
    return float((viol ** 2).mean())


def kernel(latent_states, latent_actions, state_space_samples,
           action_space_samples, _want_results=False, _trace=False):
    latent_states = np.asarray(latent_states, np.float32)
    latent_actions = np.asarray(latent_actions, np.float32)
    state_space_samples = np.asarray(state_space_samples, np.float32)
    action_space_samples = np.asarray(action_space_samples, np.float32)

    nc = _get_nc()
    in_maps, host = _make_in_maps(latent_states, latent_actions,
                                  state_space_samples, action_space_samples)
    res = run_bass_kernel_spmd(nc, in_maps, core_ids=list(range(8)),
                               trace=_trace)
    total = np.float64(0)
    total += _size_loss_host(latent_states)
    total += _size_loss_host(latent_actions)
    total += _cov_loss_host(res.results, host, "s", state_space_samples,
                            latent_states)
    total += _cov_loss_host(res.results, host, "a", action_space_samples,
                            latent_actions)
    out = np.float32(total)
    if _want_results:
        return out, res
    return out
